# revision 9
# baseline (speedup 1.0000x reference)
"""Multi-head attention (B=4, S=2048, D=1024, H=16, Hd=64) on 8 TRN2 NeuronCores.

Sharding: tensor-parallel over heads — 2 heads per core (128 channels).
Each core computes its heads' Q/K/V projections, attention, and the partial
output projection (its 128 rows of Wo); the host sums the 8 partials + bo.

Device-side layout choices:
  - x is pre-transposed on host to xT [D, B*S] so projections stream
    xT tiles as the matmul moving operand.
  - Q, K are produced transposed: QT/KT [128ch, B*S] (heads stacked on
    partitions: head0 = rows 0:64, head1 = rows 64:128). head1's score
    matmuls run at base partition 64 => row groups 2-3, packing the
    128x128 PE array with both heads concurrently (K=64 each).
  - V is produced natural [seq, ch] with a ones-column appended per head;
    the attention output matmul OT[65, q] = V_aug.T @ P then carries the
    softmax denominator in row 64 for free.
  - No max-subtraction in softmax: scores ~ N(0,1) by construction
    (|score| < ~7), exp() is safe in fp32.
"""
import sys

sys.path.insert(0, "/opt/trn_rl_repo")

import numpy as np
import ml_dtypes

import concourse.bass as bass
import concourse.mybir as mybir
import concourse.tile as tile
from concourse import bacc, bass_utils

B, S, D = 4, 2048, 1024
BS = B * S            # 8192 rows
NCORES = 8
CPC = 128             # channels per core (2 heads x 64)
HD = 64               # head dim
P = 128
QT_TILE = 512         # q-tile width
NQT = BS // QT_TILE   # 16
NKT = S // P          # 16 k-tiles per batch
NQA = S // QT_TILE    # 4 q-tiles per batch

F32 = mybir.dt.float32
CD = mybir.dt.bfloat16          # compute dtype on device
CD_NP = ml_dtypes.bfloat16

LAST_RESULTS = None
_NC_CACHE = {}


def build_nc():
    if "nc" in _NC_CACHE:
        return _NC_CACHE["nc"]
    nc = bacc.Bacc(trn_type="TRN2", num_devices=NCORES)

    xT = nc.dram_tensor("xT", [D, BS], CD, kind="ExternalInput").ap()
    wq = nc.dram_tensor("wq", [D, CPC], CD, kind="ExternalInput").ap()
    wk = nc.dram_tensor("wk", [D, CPC], CD, kind="ExternalInput").ap()
    wv = nc.dram_tensor("wv", [D, CPC], CD, kind="ExternalInput").ap()
    wo = nc.dram_tensor("wo", [CPC, D], CD, kind="ExternalInput").ap()
    bq = nc.dram_tensor("bq", [1, CPC], CD, kind="ExternalInput").ap()
    bk = nc.dram_tensor("bk", [1, CPC], CD, kind="ExternalInput").ap()
    bv = nc.dram_tensor("bv", [1, CPC], CD, kind="ExternalInput").ap()
    y = nc.dram_tensor("y", [BS, D], F32, kind="ExternalOutput").ap()

    KCH = D // P  # 8 contraction chunks for the projections

    with tile.TileContext(nc) as tc:
        with (
            tc.tile_pool(name="pers", bufs=1) as pers,
            tc.tile_pool(name="xin", bufs=2) as xin,
            tc.tile_pool(name="pt", bufs=4) as pt,
            tc.tile_pool(name="otn", bufs=3) as otn_pool,
            tc.tile_pool(name="yp", bufs=3) as yp,
            tc.tile_pool(name="sm", bufs=4) as sm,
            tc.tile_pool(name="ps", bufs=8, space="PSUM") as ps,
        ):
            # ---- persistent tensors ----
            qt_sb = pers.tile([P, BS], CD, tag="QT")
            kt_sb = pers.tile([P, BS], CD, tag="KT")
            v_sb = pers.tile([P, BS // P, 2 * HD + 2], CD, tag="V")
            wq_sb = pers.tile([P, KCH, CPC], CD, tag="wq")
            wk_sb = pers.tile([P, KCH, CPC], CD, tag="wk")
            wv_sb = pers.tile([P, KCH, CPC], CD, tag="wv")
            wo_sb = pers.tile([P, D], CD, tag="wo")
            bq_sb = pers.tile([1, CPC], CD, tag="bq")
            bk_sb = pers.tile([1, CPC], CD, tag="bk")
            bv_sb = pers.tile([1, CPC], CD, tag="bv")
            ones_sb = pers.tile([1, QT_TILE], CD, tag="ones")
            onesf_sb = pers.tile([1, HD], F32, tag="onesf")

            nc.sync.dma_start(wq_sb[:], wq.rearrange("(o p) c -> p o c", p=P))
            nc.sync.dma_start(wk_sb[:], wk.rearrange("(o p) c -> p o c", p=P))
            nc.sync.dma_start(wv_sb[:], wv.rearrange("(o p) c -> p o c", p=P))
            nc.sync.dma_start(wo_sb[:], wo[:, :])
            nc.sync.dma_start(bq_sb[:], bq[:, :])
            nc.sync.dma_start(bk_sb[:], bk[:, :])
            nc.sync.dma_start(bv_sb[:], bv[:, :])
            nc.vector.memset(ones_sb[:], 1.0)
            nc.vector.memset(onesf_sb[:], 1.0)

            # ---- phase 1: projections ----
            # stream xT in 4 big quarter-chunks (fewer DMAs => fewer
            # cross-queue sync deps); per 512-row tile compute QT/KT
            # (transposed) and V (natural); biases via K=1 ones matmuls.
            XQ = BS // 4  # 2048 rows per chunk
            for xq in range(4):
                xt = xin.tile([P, KCH, XQ], CD, tag="xt")
                nc.sync.dma_start(
                    xt[:],
                    xT[:, xq * XQ : (xq + 1) * XQ].rearrange("(o p) q -> p o q", p=P),
                )
                for lq in range(XQ // QT_TILE):
                    q0 = xq * XQ + lq * QT_TILE
                    l0 = lq * QT_TILE
                    for w_sb, b_sb, dst in (
                        (wq_sb, bq_sb, qt_sb),
                        (wk_sb, bk_sb, kt_sb),
                    ):
                        pj = ps.tile([P, QT_TILE], F32, tag="ps")
                        nc.tensor.matmul(pj[:], b_sb[0:1, :], ones_sb[0:1, :],
                                         start=True, stop=False)
                        for o in range(KCH):
                            nc.tensor.matmul(
                                pj[:], w_sb[:, o, :], xt[:, o, l0 : l0 + QT_TILE],
                                start=False, stop=(o == KCH - 1),
                            )
                        nc.vector.tensor_copy(
                            out=dst[:, q0 : q0 + QT_TILE], in_=pj[:]
                        )
                    for rt in range(QT_TILE // P):
                        vp = ps.tile([P, CPC], F32, tag="ps")
                        nc.tensor.matmul(vp[:], ones_sb[0:1, 0:P], bv_sb[0:1, :],
                                         start=True, stop=False)
                        for o in range(KCH):
                            nc.tensor.matmul(
                                vp[:],
                                xt[:, o, l0 + rt * P : l0 + (rt + 1) * P],
                                wv_sb[:, o, :],
                                start=False, stop=(o == KCH - 1),
                            )
                        grt = q0 // P + rt
                        nc.vector.tensor_copy(out=v_sb[:, grt, 0:HD], in_=vp[:, 0:HD])
                        nc.vector.tensor_copy(
                            out=v_sb[:, grt, HD + 1 : 2 * HD + 1], in_=vp[:, HD:CPC]
                        )
                        nc.vector.memset(v_sb[:, grt, HD : HD + 1], 1.0)
                        nc.vector.memset(v_sb[:, grt, 2 * HD + 1 : 2 * HD + 2], 1.0)

            # ---- phase 2+3: attention + output projection, per (batch, q-tile) ----
            scale = float(1.0 / np.sqrt(np.float32(HD)))
            for b in range(B):
                for qa in range(NQA):
                    q0 = b * S + qa * QT_TILE
                    ot = [
                        ps.tile([HD + 1, QT_TILE], F32, tag="ps", name=f"ot{h}")
                        for h in range(2)
                    ]
                    for kt in range(NKT):
                        krow = b * NKT + kt
                        k0 = b * S + kt * P
                        for h in range(2):
                            hp = h * HD
                            st = ps.tile([P, QT_TILE], F32, tag="ps")
                            nc.tensor.matmul(
                                st[:],
                                kt_sb[hp : hp + HD, k0 : k0 + P],
                                qt_sb[hp : hp + HD, q0 : q0 + QT_TILE],
                                start=True, stop=True,
                            )
                            p_t = pt.tile([P, QT_TILE], CD, tag="p")
                            nc.scalar.activation(
                                p_t[:], st[:], mybir.ActivationFunctionType.Exp,
                                scale=scale,
                            )
                            vcol = h * (HD + 1)
                            nc.tensor.matmul(
                                ot[h][:],
                                v_sb[:, krow, vcol : vcol + HD + 1],
                                p_t[:],
                                start=(kt == 0), stop=(kt == NKT - 1),
                            )
                    # softmax normalization: row 64 of ot[h] holds the denominators
                    sums = sm.tile([1, 2 * QT_TILE], F32, tag="sums")
                    nc.vector.tensor_copy(
                        out=sums[0:1, 0:QT_TILE], in_=ot[0][HD : HD + 1, :]
                    )
                    nc.vector.tensor_copy(
                        out=sums[0:1, QT_TILE:], in_=ot[1][HD : HD + 1, :]
                    )
                    recip = sm.tile([1, 2 * QT_TILE], F32, tag="recip")
                    nc.vector.reciprocal(recip[:], sums[:])
                    # broadcast 1/sum across the 64 head channels (K=1 matmuls)
                    r_sb = sm.tile([P, QT_TILE], F32, tag="rsb")
                    for h in range(2):
                        rps = ps.tile([HD, QT_TILE], F32, tag="ps", name=f"rps{h}")
                        nc.tensor.matmul(
                            rps[:],
                            onesf_sb[0:1, :],
                            recip[0:1, h * QT_TILE : (h + 1) * QT_TILE],
                            start=True, stop=True,
                        )
                        nc.vector.tensor_copy(
                            out=r_sb[h * HD : (h + 1) * HD, :], in_=rps[:]
                        )
                    # combined normalized attention output [128ch, 512q] (bf16)
                    on = otn_pool.tile([P, QT_TILE], CD, tag="otn")
                    nc.vector.tensor_mul(
                        out=on[0:HD, :], in0=ot[0][0:HD, :], in1=r_sb[0:HD, :]
                    )
                    nc.vector.tensor_mul(
                        out=on[HD:CPC, :], in0=ot[1][0:HD, :], in1=r_sb[HD:CPC, :]
                    )
                    # output projection: y[q0:q0+512, :] partial = on.T @ wo
                    for j in range(QT_TILE // P):
                        ysb = yp.tile([P, D], F32, tag="y")
                        for e in range(D // QT_TILE):
                            yps = ps.tile([P, QT_TILE], F32, tag="ps")
                            nc.tensor.matmul(
                                yps[:],
                                on[:, j * P : (j + 1) * P],
                                wo_sb[:, e * QT_TILE : (e + 1) * QT_TILE],
                                start=True, stop=True,
                            )
                            nc.vector.tensor_copy(
                                out=ysb[:, e * QT_TILE : (e + 1) * QT_TILE], in_=yps[:]
                            )
                        nc.sync.dma_start(
                            y[q0 + j * P : q0 + (j + 1) * P, :], ysb[:]
                        )
    nc.compile()
    _NC_CACHE["nc"] = nc
    return nc


def make_in_maps(inputs):
    x = np.asarray(inputs["x"], np.float32)
    Wq = np.asarray(inputs["Wq"], np.float32)
    Wk = np.asarray(inputs["Wk"], np.float32)
    Wv = np.asarray(inputs["Wv"], np.float32)
    Wo = np.asarray(inputs["Wo"], np.float32)
    bq = np.asarray(inputs["bq"], np.float32)
    bk = np.asarray(inputs["bk"], np.float32)
    bv = np.asarray(inputs["bv"], np.float32)

    xT = np.ascontiguousarray(x.reshape(BS, D).T).astype(CD_NP)
    in_maps = []
    for c in range(NCORES):
        sl = slice(c * CPC, (c + 1) * CPC)
        in_maps.append(
            {
                "xT": xT,
                "wq": np.ascontiguousarray(Wq[:, sl]).astype(CD_NP),
                "wk": np.ascontiguousarray(Wk[:, sl]).astype(CD_NP),
                "wv": np.ascontiguousarray(Wv[:, sl]).astype(CD_NP),
                "wo": np.ascontiguousarray(Wo[sl, :]).astype(CD_NP),
                "bq": np.ascontiguousarray(bq[sl].reshape(1, CPC)).astype(CD_NP),
                "bk": np.ascontiguousarray(bk[sl].reshape(1, CPC)).astype(CD_NP),
                "bv": np.ascontiguousarray(bv[sl].reshape(1, CPC)).astype(CD_NP),
            }
        )
    return in_maps


def kernel(**inputs):
    global LAST_RESULTS
    bo = np.asarray(inputs["bo"], np.float32)
    nc = build_nc()
    in_maps = make_in_maps(inputs)
    res = bass_utils.run_bass_kernel_spmd(nc, in_maps, core_ids=list(range(NCORES)))
    LAST_RESULTS = res
    acc = np.zeros((BS, D), np.float64)
    for r in res.results:
        acc += r["y"].astype(np.float64)
    out = (acc + bo.astype(np.float64)).astype(np.float32)
    return out.reshape(B, S, D)


# revision 10
# speedup vs baseline: 1.4955x; 1.4955x over previous
"""Multi-head attention (B=4, S=2048, D=1024, H=16, Hd=64) on 8 TRN2 NeuronCores.

Sharding: tensor-parallel over heads — 2 heads per core (128 channels).
Each core computes its heads' Q/K/V projections, attention, and the partial
output projection (its 128 rows of Wo); the host sums the 8 partials + bo.

Device-side layout choices:
  - x is pre-transposed on host to xT [D, B*S] so projections stream
    xT tiles as the matmul moving operand.
  - Q, K are produced transposed: QT/KT [128ch, B*S] (heads stacked on
    partitions: head0 = rows 0:64, head1 = rows 64:128). head1's score
    matmuls run at base partition 64 => row groups 2-3, packing the
    128x128 PE array with both heads concurrently (K=64 each).
  - V is produced natural [seq, ch] with a ones-column appended per head;
    the attention output matmul OT[65, q] = V_aug.T @ P then carries the
    softmax denominator in row 64 for free.
  - No max-subtraction in softmax: scores ~ N(0,1) by construction
    (|score| < ~7), exp() is safe in fp32.
"""
import sys

sys.path.insert(0, "/opt/trn_rl_repo")

import numpy as np
import ml_dtypes

import concourse.bass as bass
import concourse.mybir as mybir
import concourse.tile as tile
from concourse import bacc, bass_utils

B, S, D = 4, 2048, 1024
BS = B * S            # 8192 rows
NCORES = 8
CPC = 128             # channels per core (2 heads x 64)
HD = 64               # head dim
P = 128
QT_TILE = 512         # q-tile width
NQT = BS // QT_TILE   # 16
NKT = S // P          # 16 k-tiles per batch
NQA = S // QT_TILE    # 4 q-tiles per batch

F32 = mybir.dt.float32
CD = mybir.dt.bfloat16          # compute dtype on device
CD_NP = ml_dtypes.bfloat16

LAST_RESULTS = None
_NC_CACHE = {}


def build_nc():
    if "nc" in _NC_CACHE:
        return _NC_CACHE["nc"]
    nc = bacc.Bacc(trn_type="TRN2", num_devices=NCORES)

    xT = nc.dram_tensor("xT", [D, BS], CD, kind="ExternalInput").ap()
    wq = nc.dram_tensor("wq", [D, CPC], CD, kind="ExternalInput").ap()
    wk = nc.dram_tensor("wk", [D, CPC], CD, kind="ExternalInput").ap()
    wv = nc.dram_tensor("wv", [D, CPC], CD, kind="ExternalInput").ap()
    wo = nc.dram_tensor("wo", [CPC, D], CD, kind="ExternalInput").ap()
    bq = nc.dram_tensor("bq", [1, CPC], CD, kind="ExternalInput").ap()
    bk = nc.dram_tensor("bk", [1, CPC], CD, kind="ExternalInput").ap()
    bv = nc.dram_tensor("bv", [1, CPC], CD, kind="ExternalInput").ap()
    y = nc.dram_tensor("y", [BS, D], F32, kind="ExternalOutput").ap()

    KCH = D // P  # 8 contraction chunks for the projections

    with tile.TileContext(nc) as tc:
        with (
            tc.tile_pool(name="pers", bufs=1) as pers,
            tc.tile_pool(name="xin", bufs=2) as xin,
            tc.tile_pool(name="pt", bufs=4) as pt,
            tc.tile_pool(name="otn", bufs=3) as otn_pool,
            tc.tile_pool(name="yp", bufs=3) as yp,
            tc.tile_pool(name="sm", bufs=4) as sm,
            tc.tile_pool(name="ps", bufs=8, space="PSUM") as ps,
        ):
            # ---- persistent tensors ----
            qt_sb = pers.tile([P, BS], CD, tag="QT")
            kt_sb = pers.tile([P, BS], CD, tag="KT")
            v_sb = pers.tile([P, BS // P, 2 * HD + 2], CD, tag="V")
            wq_sb = pers.tile([P, KCH, CPC], CD, tag="wq")
            wk_sb = pers.tile([P, KCH, CPC], CD, tag="wk")
            wv_sb = pers.tile([P, KCH, CPC], CD, tag="wv")
            wo_sb = pers.tile([P, D], CD, tag="wo")
            bq_sb = pers.tile([1, CPC], CD, tag="bq")
            bk_sb = pers.tile([1, CPC], CD, tag="bk")
            bv_sb = pers.tile([1, CPC], CD, tag="bv")
            ones_sb = pers.tile([1, QT_TILE], CD, tag="ones")
            onesf_sb = pers.tile([1, HD], F32, tag="onesf")

            nc.sync.dma_start(wq_sb[:], wq.rearrange("(o p) c -> p o c", p=P))
            nc.sync.dma_start(wk_sb[:], wk.rearrange("(o p) c -> p o c", p=P))
            nc.sync.dma_start(wv_sb[:], wv.rearrange("(o p) c -> p o c", p=P))
            nc.sync.dma_start(wo_sb[:], wo[:, :])
            nc.sync.dma_start(bq_sb[:], bq[:, :])
            nc.sync.dma_start(bk_sb[:], bk[:, :])
            nc.sync.dma_start(bv_sb[:], bv[:, :])
            nc.vector.memset(ones_sb[:], 1.0)
            nc.vector.memset(onesf_sb[:], 1.0)

            # ---- phase 1: projections ----
            # stream xT in 4 big quarter-chunks (fewer DMAs => fewer
            # cross-queue sync deps); per 512-row tile compute QT/KT
            # (transposed) and V (natural); biases via K=1 ones matmuls.
            XQ = BS // 4  # 2048 rows per chunk
            for xq in range(4):
                xt = xin.tile([P, KCH, XQ], CD, tag="xt")
                nc.sync.dma_start(
                    xt[:],
                    xT[:, xq * XQ : (xq + 1) * XQ].rearrange("(o p) q -> p o q", p=P),
                )
                for lq in range(XQ // QT_TILE):
                    q0 = xq * XQ + lq * QT_TILE
                    l0 = lq * QT_TILE
                    for w_sb, b_sb, dst in (
                        (wq_sb, bq_sb, qt_sb),
                        (wk_sb, bk_sb, kt_sb),
                    ):
                        pj = ps.tile([P, QT_TILE], F32, tag="ps")
                        nc.tensor.matmul(pj[:], b_sb[0:1, :], ones_sb[0:1, :],
                                         start=True, stop=False)
                        for o in range(KCH):
                            nc.tensor.matmul(
                                pj[:], w_sb[:, o, :], xt[:, o, l0 : l0 + QT_TILE],
                                start=False, stop=(o == KCH - 1),
                            )
                        nc.vector.tensor_copy(
                            out=dst[:, q0 : q0 + QT_TILE], in_=pj[:]
                        )
                    for rt in range(QT_TILE // P):
                        vp = ps.tile([P, CPC], F32, tag="ps")
                        nc.tensor.matmul(vp[:], ones_sb[0:1, 0:P], bv_sb[0:1, :],
                                         start=True, stop=False)
                        for o in range(KCH):
                            nc.tensor.matmul(
                                vp[:],
                                xt[:, o, l0 + rt * P : l0 + (rt + 1) * P],
                                wv_sb[:, o, :],
                                start=False, stop=(o == KCH - 1),
                            )
                        grt = q0 // P + rt
                        nc.vector.tensor_copy(out=v_sb[:, grt, 0:HD], in_=vp[:, 0:HD])
                        nc.vector.tensor_copy(
                            out=v_sb[:, grt, HD + 1 : 2 * HD + 1], in_=vp[:, HD:CPC]
                        )
                        nc.vector.memset(v_sb[:, grt, HD : HD + 1], 1.0)
                        nc.vector.memset(v_sb[:, grt, 2 * HD + 1 : 2 * HD + 2], 1.0)

            # ---- phase 2+3: attention + output projection, per (batch, q-tile) ----
            # Software-pipelined: the AV matmuls run one k-step behind the
            # score matmuls, so the PE never waits on ACT's exp, and the two
            # heads' K=64 score matmuls are adjacent (disjoint row groups =>
            # they pack the 128-row PE array concurrently).
            scale = float(1.0 / np.sqrt(np.float32(HD)))
            for b in range(B):
                for qa in range(NQA):
                    q0 = b * S + qa * QT_TILE
                    ot = [
                        ps.tile([HD + 1, QT_TILE], F32, tag="ps", name=f"ot{h}")
                        for h in range(2)
                    ]

                    def emit_st_exp(kt, q0=q0, b=b):
                        k0 = b * S + kt * P
                        pair = []
                        sts = []
                        for h in range(2):
                            hp = h * HD
                            st = ps.tile([P, QT_TILE], F32, tag="ps", name=f"st{h}")
                            nc.tensor.matmul(
                                st[:],
                                kt_sb[hp : hp + HD, k0 : k0 + P],
                                qt_sb[hp : hp + HD, q0 : q0 + QT_TILE],
                                start=True, stop=True,
                            )
                            sts.append(st)
                        for h in range(2):
                            p_t = pt.tile([P, QT_TILE], CD, tag="p", name=f"p{h}")
                            nc.scalar.activation(
                                p_t[:], sts[h][:], mybir.ActivationFunctionType.Exp,
                                scale=scale,
                            )
                            pair.append(p_t)
                        return pair

                    def emit_av(kt, pair, b=b):
                        krow = b * NKT + kt
                        for h in range(2):
                            vcol = h * (HD + 1)
                            nc.tensor.matmul(
                                ot[h][:],
                                v_sb[:, krow, vcol : vcol + HD + 1],
                                pair[h][:],
                                start=(kt == 0), stop=(kt == NKT - 1),
                            )

                    prev = emit_st_exp(0)
                    for kt in range(1, NKT):
                        cur = emit_st_exp(kt)
                        emit_av(kt - 1, prev)
                        prev = cur
                    emit_av(NKT - 1, prev)

                    # softmax normalization: row 64 of ot[h] holds the
                    # denominators. Broadcast the sums across 64 partitions
                    # via K=1 matmul FIRST, then reciprocal at full width
                    # (a [1,N] reciprocal runs on a single DVE lane - 6.5us).
                    sums = sm.tile([1, 2 * QT_TILE], F32, tag="sums")
                    nc.vector.tensor_copy(
                        out=sums[0:1, 0:QT_TILE], in_=ot[0][HD : HD + 1, :]
                    )
                    nc.vector.tensor_copy(
                        out=sums[0:1, QT_TILE:], in_=ot[1][HD : HD + 1, :]
                    )
                    rps = ps.tile([P, QT_TILE], F32, tag="ps", name="rps")
                    for h in range(2):
                        nc.tensor.matmul(
                            rps[h * HD : (h + 1) * HD, :],
                            onesf_sb[0:1, :],
                            sums[0:1, h * QT_TILE : (h + 1) * QT_TILE],
                            start=True, stop=True,
                        )
                    r_sb = sm.tile([P, QT_TILE], F32, tag="rsb")
                    nc.vector.reciprocal(r_sb[:], rps[:])
                    # combined normalized attention output [128ch, 512q] (bf16)
                    on = otn_pool.tile([P, QT_TILE], CD, tag="otn")
                    nc.vector.tensor_mul(
                        out=on[0:HD, :], in0=ot[0][0:HD, :], in1=r_sb[0:HD, :]
                    )
                    nc.vector.tensor_mul(
                        out=on[HD:CPC, :], in0=ot[1][0:HD, :], in1=r_sb[HD:CPC, :]
                    )
                    # output projection: y[q0:q0+512, :] partial = on.T @ wo
                    for j in range(QT_TILE // P):
                        ysb = yp.tile([P, D], F32, tag="y")
                        for e in range(D // QT_TILE):
                            yps = ps.tile([P, QT_TILE], F32, tag="ps")
                            nc.tensor.matmul(
                                yps[:],
                                on[:, j * P : (j + 1) * P],
                                wo_sb[:, e * QT_TILE : (e + 1) * QT_TILE],
                                start=True, stop=True,
                            )
                            nc.vector.tensor_copy(
                                out=ysb[:, e * QT_TILE : (e + 1) * QT_TILE], in_=yps[:]
                            )
                        nc.sync.dma_start(
                            y[q0 + j * P : q0 + (j + 1) * P, :], ysb[:]
                        )
    nc.compile()
    _NC_CACHE["nc"] = nc
    return nc


def make_in_maps(inputs):
    x = np.asarray(inputs["x"], np.float32)
    Wq = np.asarray(inputs["Wq"], np.float32)
    Wk = np.asarray(inputs["Wk"], np.float32)
    Wv = np.asarray(inputs["Wv"], np.float32)
    Wo = np.asarray(inputs["Wo"], np.float32)
    bq = np.asarray(inputs["bq"], np.float32)
    bk = np.asarray(inputs["bk"], np.float32)
    bv = np.asarray(inputs["bv"], np.float32)

    xT = np.ascontiguousarray(x.reshape(BS, D).T).astype(CD_NP)
    in_maps = []
    for c in range(NCORES):
        sl = slice(c * CPC, (c + 1) * CPC)
        in_maps.append(
            {
                "xT": xT,
                "wq": np.ascontiguousarray(Wq[:, sl]).astype(CD_NP),
                "wk": np.ascontiguousarray(Wk[:, sl]).astype(CD_NP),
                "wv": np.ascontiguousarray(Wv[:, sl]).astype(CD_NP),
                "wo": np.ascontiguousarray(Wo[sl, :]).astype(CD_NP),
                "bq": np.ascontiguousarray(bq[sl].reshape(1, CPC)).astype(CD_NP),
                "bk": np.ascontiguousarray(bk[sl].reshape(1, CPC)).astype(CD_NP),
                "bv": np.ascontiguousarray(bv[sl].reshape(1, CPC)).astype(CD_NP),
            }
        )
    return in_maps


def kernel(**inputs):
    global LAST_RESULTS
    bo = np.asarray(inputs["bo"], np.float32)
    nc = build_nc()
    in_maps = make_in_maps(inputs)
    res = bass_utils.run_bass_kernel_spmd(nc, in_maps, core_ids=list(range(NCORES)))
    LAST_RESULTS = res
    acc = np.zeros((BS, D), np.float64)
    for r in res.results:
        acc += r["y"].astype(np.float64)
    out = (acc + bo.astype(np.float64)).astype(np.float32)
    return out.reshape(B, S, D)


# revision 11
# speedup vs baseline: 1.6359x; 1.0939x over previous
"""Multi-head attention (B=4, S=2048, D=1024, H=16, Hd=64) on 8 TRN2 NeuronCores.

Sharding: tensor-parallel over heads — 2 heads per core (128 channels).
Each core computes its heads' Q/K/V projections, attention, and the partial
output projection (its 128 rows of Wo); the host sums the 8 partials + bo.

Device-side structure (per core):
  - x is pre-transposed on host to xT [D, B*S]; streamed in 4 big chunks.
  - Q, K produced transposed: QT/KT [128ch, B*S], heads stacked on
    partitions (head0 rows 0:64, head1 rows 64:128). The two heads' K=64
    score matmuls are emitted adjacently at disjoint row groups, so they
    run concurrently in the 128x128 PE array.
  - V is computed transposed (VT, N=512 matmuls) then PE-transposed into
    natural [seq, ch] layout with a ones-column per head; the attention
    output matmul OT[65, q] = V_aug.T @ P carries the softmax denominator
    in row 64 for free.
  - Both heads' score tiles share one 2-bank PSUM tile, so exp() runs as
    a single 1024-wide ACT op (half the ACT instruction count).
  - Attention is software-pipelined: AV matmuls lag the score matmuls by
    2 k-steps, and the normalization + output projection of block i is
    emitted inside block i+1's first score matmuls, so the PE never
    stalls on the ACT/DVE chains.
  - No max-subtraction in softmax: scores ~ N(0,1) by construction
    (|score| < ~7), exp() is safe in fp32.
"""
import sys

sys.path.insert(0, "/opt/trn_rl_repo")

import numpy as np
import ml_dtypes

import concourse.bass as bass
import concourse.mybir as mybir
import concourse.tile as tile
from concourse import bacc, bass_utils
from concourse.masks import make_identity

B, S, D = 4, 2048, 1024
BS = B * S            # 8192 rows
NCORES = 8
CPC = 128             # channels per core (2 heads x 64)
HD = 64               # head dim
P = 128
QT_TILE = 512         # q-tile width
NQT = BS // QT_TILE   # 16
NKT = S // P          # 16 k-tiles per batch
NQA = S // QT_TILE    # 4 q-tiles per batch

F32 = mybir.dt.float32
CD = mybir.dt.bfloat16          # compute dtype on device
CD_NP = ml_dtypes.bfloat16

LAST_RESULTS = None
_NC_CACHE = {}


def build_nc():
    if "nc" in _NC_CACHE:
        return _NC_CACHE["nc"]
    nc = bacc.Bacc(trn_type="TRN2", num_devices=NCORES)

    xT = nc.dram_tensor("xT", [D, BS], CD, kind="ExternalInput").ap()
    wq = nc.dram_tensor("wq", [D, CPC], CD, kind="ExternalInput").ap()
    wk = nc.dram_tensor("wk", [D, CPC], CD, kind="ExternalInput").ap()
    wv = nc.dram_tensor("wv", [D, CPC], CD, kind="ExternalInput").ap()
    wo = nc.dram_tensor("wo", [CPC, D], CD, kind="ExternalInput").ap()
    bq = nc.dram_tensor("bq", [1, CPC], CD, kind="ExternalInput").ap()
    bk = nc.dram_tensor("bk", [1, CPC], CD, kind="ExternalInput").ap()
    bv = nc.dram_tensor("bv", [1, CPC], CD, kind="ExternalInput").ap()
    y = nc.dram_tensor("y", [BS, D], F32, kind="ExternalOutput").ap()

    KCH = D // P  # 8 contraction chunks for the projections
    scale = float(1.0 / np.sqrt(np.float32(HD)))

    with tile.TileContext(nc) as tc:
        with (
            tc.tile_pool(name="pers", bufs=1) as pers,
            tc.tile_pool(name="xin", bufs=2) as xin,
            tc.tile_pool(name="vtp", bufs=2) as vtp,
            tc.tile_pool(name="pt", bufs=3) as pt,
            tc.tile_pool(name="otn", bufs=2) as otn_pool,
            tc.tile_pool(name="yp", bufs=3) as yp,
            tc.tile_pool(name="sm", bufs=4) as sm,
            tc.tile_pool(name="ps1", bufs=4, space="PSUM") as ps1,
            tc.tile_pool(name="ps2", bufs=2, space="PSUM") as ps2,
        ):
            # ---- persistent tensors ----
            qt_sb = pers.tile([P, BS], CD, tag="QT")
            kt_sb = pers.tile([P, BS], CD, tag="KT")
            v_sb = pers.tile([P, BS // P, 2 * HD + 2], CD, tag="V")
            wq_sb = pers.tile([P, KCH, CPC], CD, tag="wq")
            wk_sb = pers.tile([P, KCH, CPC], CD, tag="wk")
            wv_sb = pers.tile([P, KCH, CPC], CD, tag="wv")
            wo_sb = pers.tile([P, D], CD, tag="wo")
            bq_sb = pers.tile([1, CPC], CD, tag="bq")
            bk_sb = pers.tile([1, CPC], CD, tag="bk")
            bv_sb = pers.tile([1, CPC], CD, tag="bv")
            ones_sb = pers.tile([1, QT_TILE], CD, tag="ones")
            onesf_sb = pers.tile([1, HD], F32, tag="onesf")
            ident_sb = pers.tile([P, P], CD, tag="ident")

            nc.sync.dma_start(wq_sb[:], wq.rearrange("(o p) c -> p o c", p=P))
            nc.sync.dma_start(wk_sb[:], wk.rearrange("(o p) c -> p o c", p=P))
            nc.sync.dma_start(wv_sb[:], wv.rearrange("(o p) c -> p o c", p=P))
            nc.sync.dma_start(wo_sb[:], wo[:, :])
            nc.sync.dma_start(bq_sb[:], bq[:, :])
            nc.sync.dma_start(bk_sb[:], bk[:, :])
            nc.sync.dma_start(bv_sb[:], bv[:, :])
            nc.vector.memset(ones_sb[:], 1.0)
            nc.vector.memset(onesf_sb[:], 1.0)
            make_identity(nc, ident_sb[:])

            # ---- phase 1: projections ----
            XQ = BS // 4  # 2048 rows per x chunk
            for xq in range(4):
                xt = xin.tile([P, KCH, XQ], CD, tag="xt")
                nc.sync.dma_start(
                    xt[:],
                    xT[:, xq * XQ : (xq + 1) * XQ].rearrange("(o p) q -> p o q", p=P),
                )
                for lq in range(XQ // QT_TILE):
                    q0 = xq * XQ + lq * QT_TILE
                    l0 = lq * QT_TILE
                    for w_sb, b_sb, dst in (
                        (wq_sb, bq_sb, qt_sb),
                        (wk_sb, bk_sb, kt_sb),
                    ):
                        pj = ps1.tile([P, QT_TILE], F32, tag="ps")
                        nc.tensor.matmul(pj[:], b_sb[0:1, :], ones_sb[0:1, :],
                                         start=True, stop=False)
                        for o in range(KCH):
                            nc.tensor.matmul(
                                pj[:], w_sb[:, o, :], xt[:, o, l0 : l0 + QT_TILE],
                                start=False, stop=(o == KCH - 1),
                            )
                        nc.vector.tensor_copy(
                            out=dst[:, q0 : q0 + QT_TILE], in_=pj[:]
                        )
                    # V: compute VT (N=512 matmuls), then PE-transpose to
                    # natural layout with the two per-head ones-columns.
                    pv = ps1.tile([P, QT_TILE], F32, tag="ps")
                    nc.tensor.matmul(pv[:], bv_sb[0:1, :], ones_sb[0:1, :],
                                     start=True, stop=False)
                    for o in range(KCH):
                        nc.tensor.matmul(
                            pv[:], wv_sb[:, o, :], xt[:, o, l0 : l0 + QT_TILE],
                            start=False, stop=(o == KCH - 1),
                        )
                    vt_sb = vtp.tile([P, QT_TILE], CD, tag="vt")
                    nc.vector.tensor_copy(out=vt_sb[:], in_=pv[:])
                    for rt in range(QT_TILE // P):
                        tp = ps1.tile([P, P], CD, tag="ps", name="tp")
                        nc.tensor.transpose(
                            tp[:], vt_sb[:, rt * P : (rt + 1) * P], ident_sb[:]
                        )
                        grt = q0 // P + rt
                        nc.vector.tensor_copy(out=v_sb[:, grt, 0:HD], in_=tp[:, 0:HD])
                        nc.vector.tensor_copy(
                            out=v_sb[:, grt, HD + 1 : 2 * HD + 1], in_=tp[:, HD:CPC]
                        )
                        nc.vector.memset(v_sb[:, grt, HD : HD + 1], 1.0)
                        nc.vector.memset(v_sb[:, grt, 2 * HD + 1 : 2 * HD + 2], 1.0)

            # ---- phase 2+3: attention + output projection ----
            def emit_st_exp(b, qa, kt):
                q0 = b * S + qa * QT_TILE
                k0 = b * S + kt * P
                stp = ps2.tile([P, 2 * QT_TILE], F32, tag="stp", name="stp")
                for h in range(2):
                    hp = h * HD
                    nc.tensor.matmul(
                        stp[:, h * QT_TILE : (h + 1) * QT_TILE],
                        kt_sb[hp : hp + HD, k0 : k0 + P],
                        qt_sb[hp : hp + HD, q0 : q0 + QT_TILE],
                        start=True, stop=True,
                    )
                p_t = pt.tile([P, 2 * QT_TILE], CD, tag="p", name="p")
                nc.scalar.activation(
                    p_t[:], stp[:], mybir.ActivationFunctionType.Exp, scale=scale
                )
                return p_t

            def emit_av(ot, b, kt, p_t):
                krow = b * NKT + kt
                for h in range(2):
                    vcol = h * (HD + 1)
                    nc.tensor.matmul(
                        ot[h][:],
                        v_sb[:, krow, vcol : vcol + HD + 1],
                        p_t[:, h * QT_TILE : (h + 1) * QT_TILE],
                        start=(kt == 0), stop=(kt == NKT - 1),
                    )

            def finalize(fin):
                b, qa, ot = fin
                q0 = b * S + qa * QT_TILE
                # softmax denominators live in row 64 of each ot tile
                sums = sm.tile([1, 2 * QT_TILE], F32, tag="sums")
                nc.vector.tensor_copy(
                    out=sums[0:1, 0:QT_TILE], in_=ot[0][HD : HD + 1, :]
                )
                nc.vector.tensor_copy(
                    out=sums[0:1, QT_TILE:], in_=ot[1][HD : HD + 1, :]
                )
                rps = ps1.tile([P, QT_TILE], F32, tag="ps", name="rps")
                for h in range(2):
                    nc.tensor.matmul(
                        rps[h * HD : (h + 1) * HD, :],
                        onesf_sb[0:1, :],
                        sums[0:1, h * QT_TILE : (h + 1) * QT_TILE],
                        start=True, stop=True,
                    )
                r_sb = sm.tile([P, QT_TILE], F32, tag="rsb")
                nc.vector.reciprocal(r_sb[:], rps[:])
                on = otn_pool.tile([P, QT_TILE], CD, tag="otn")
                nc.vector.tensor_mul(
                    out=on[0:HD, :], in0=ot[0][0:HD, :], in1=r_sb[0:HD, :]
                )
                nc.vector.tensor_mul(
                    out=on[HD:CPC, :], in0=ot[1][0:HD, :], in1=r_sb[HD:CPC, :]
                )
                # output projection: y[q0:q0+512, :] partial = on.T @ wo
                for j in range(QT_TILE // P):
                    ysb = yp.tile([P, D], F32, tag="y")
                    for e in range(D // QT_TILE):
                        yps = ps1.tile([P, QT_TILE], F32, tag="ps", name="yps")
                        nc.tensor.matmul(
                            yps[:],
                            on[:, j * P : (j + 1) * P],
                            wo_sb[:, e * QT_TILE : (e + 1) * QT_TILE],
                            start=True, stop=True,
                        )
                        nc.vector.tensor_copy(
                            out=ysb[:, e * QT_TILE : (e + 1) * QT_TILE], in_=yps[:]
                        )
                    nc.sync.dma_start(y[q0 + j * P : q0 + (j + 1) * P, :], ysb[:])

            blocks = [(b, qa) for b in range(B) for qa in range(NQA)]
            prev_fin = None
            for b, qa in blocks:
                pts = {0: emit_st_exp(b, qa, 0), 1: emit_st_exp(b, qa, 1)}
                if prev_fin is not None:
                    finalize(prev_fin)
                    prev_fin = None
                ot = [
                    ps1.tile([HD + 1, QT_TILE], F32, tag="ps", name=f"ot{h}")
                    for h in range(2)
                ]
                for kt in range(2, NKT):
                    pts[kt] = emit_st_exp(b, qa, kt)
                    emit_av(ot, b, kt - 2, pts.pop(kt - 2))
                emit_av(ot, b, NKT - 2, pts.pop(NKT - 2))
                emit_av(ot, b, NKT - 1, pts.pop(NKT - 1))
                prev_fin = (b, qa, ot)
            finalize(prev_fin)

    nc.compile()
    _NC_CACHE["nc"] = nc
    return nc


def make_in_maps(inputs):
    x = np.asarray(inputs["x"], np.float32)
    Wq = np.asarray(inputs["Wq"], np.float32)
    Wk = np.asarray(inputs["Wk"], np.float32)
    Wv = np.asarray(inputs["Wv"], np.float32)
    Wo = np.asarray(inputs["Wo"], np.float32)
    bq = np.asarray(inputs["bq"], np.float32)
    bk = np.asarray(inputs["bk"], np.float32)
    bv = np.asarray(inputs["bv"], np.float32)

    xT = np.ascontiguousarray(x.reshape(BS, D).T).astype(CD_NP)
    in_maps = []
    for c in range(NCORES):
        sl = slice(c * CPC, (c + 1) * CPC)
        in_maps.append(
            {
                "xT": xT,
                "wq": np.ascontiguousarray(Wq[:, sl]).astype(CD_NP),
                "wk": np.ascontiguousarray(Wk[:, sl]).astype(CD_NP),
                "wv": np.ascontiguousarray(Wv[:, sl]).astype(CD_NP),
                "wo": np.ascontiguousarray(Wo[sl, :]).astype(CD_NP),
                "bq": np.ascontiguousarray(bq[sl].reshape(1, CPC)).astype(CD_NP),
                "bk": np.ascontiguousarray(bk[sl].reshape(1, CPC)).astype(CD_NP),
                "bv": np.ascontiguousarray(bv[sl].reshape(1, CPC)).astype(CD_NP),
            }
        )
    return in_maps


def kernel(**inputs):
    global LAST_RESULTS
    bo = np.asarray(inputs["bo"], np.float32)
    nc = build_nc()
    in_maps = make_in_maps(inputs)
    res = bass_utils.run_bass_kernel_spmd(nc, in_maps, core_ids=list(range(NCORES)))
    LAST_RESULTS = res
    acc = np.zeros((BS, D), np.float64)
    for r in res.results:
        acc += r["y"].astype(np.float64)
    out = (acc + bo.astype(np.float64)).astype(np.float32)
    return out.reshape(B, S, D)


# revision 13
# speedup vs baseline: 1.6756x; 1.0243x over previous
"""Multi-head attention (B=4, S=2048, D=1024, H=16, Hd=64) on 8 TRN2 NeuronCores.

Sharding: tensor-parallel over heads — 2 heads per core (128 channels).
Each core computes its heads' Q/K/V projections, attention, and the partial
output projection (its 128 rows of Wo); the host sums the 8 partials + bo.

Device-side structure (per core):
  - x is pre-transposed on host to xT [D, B*S]; streamed in 4 big chunks.
  - Q, K produced transposed: QT/KT [128ch, B*S], heads stacked on
    partitions (head0 rows 0:64, head1 rows 64:128). The two heads' K=64
    score matmuls are emitted adjacently at disjoint row groups, so they
    run concurrently in the 128x128 PE array.
  - V is computed transposed (VT, N=512 matmuls) then PE-transposed into
    natural [seq, ch] layout with a ones-column per head; the attention
    output matmul OT[65, q] = V_aug.T @ P carries the softmax denominator
    in row 64 for free.
  - Both heads' score tiles share one 2-bank PSUM tile, so exp() runs as
    a single 1024-wide ACT op (half the ACT instruction count).
  - Attention is software-pipelined: AV matmuls lag the score matmuls by
    2 k-steps, and the normalization + output projection of block i is
    emitted inside block i+1's first score matmuls, so the PE never
    stalls on the ACT/DVE chains.
  - No max-subtraction in softmax: scores ~ N(0,1) by construction
    (|score| < ~7), exp() is safe in fp32.
"""
import sys

sys.path.insert(0, "/opt/trn_rl_repo")

import numpy as np
import ml_dtypes

import concourse.bass as bass
import concourse.mybir as mybir
import concourse.tile as tile
from concourse import bacc, bass_utils
from concourse.masks import make_identity

B, S, D = 4, 2048, 1024
BS = B * S            # 8192 rows
NCORES = 8
CPC = 128             # channels per core (2 heads x 64)
HD = 64               # head dim
P = 128
QT_TILE = 512         # q-tile width
NQT = BS // QT_TILE   # 16
NKT = S // P          # 16 k-tiles per batch
NQA = S // QT_TILE    # 4 q-tiles per batch

F32 = mybir.dt.float32
CD = mybir.dt.bfloat16          # compute dtype on device
CD_NP = ml_dtypes.bfloat16

LAST_RESULTS = None
_NC_CACHE = {}


def build_nc():
    if "nc" in _NC_CACHE:
        return _NC_CACHE["nc"]
    nc = bacc.Bacc(trn_type="TRN2", num_devices=NCORES)

    xT = nc.dram_tensor("xT", [D, BS], CD, kind="ExternalInput").ap()
    wq = nc.dram_tensor("wq", [D, CPC], CD, kind="ExternalInput").ap()
    wk = nc.dram_tensor("wk", [D, CPC], CD, kind="ExternalInput").ap()
    wv = nc.dram_tensor("wv", [D, CPC], CD, kind="ExternalInput").ap()
    wo = nc.dram_tensor("wo", [CPC, D], CD, kind="ExternalInput").ap()
    bq = nc.dram_tensor("bq", [1, CPC], CD, kind="ExternalInput").ap()
    bk = nc.dram_tensor("bk", [1, CPC], CD, kind="ExternalInput").ap()
    bv = nc.dram_tensor("bv", [1, CPC], CD, kind="ExternalInput").ap()
    y = nc.dram_tensor("y", [BS, D], F32, kind="ExternalOutput").ap()

    KCH = D // P  # 8 contraction chunks for the projections
    scale = float(1.0 / np.sqrt(np.float32(HD)))

    with tile.TileContext(nc) as tc:
        with (
            tc.tile_pool(name="pers", bufs=1) as pers,
            tc.tile_pool(name="xin", bufs=2) as xin,
            tc.tile_pool(name="vtp", bufs=2) as vtp,
            tc.tile_pool(name="pt", bufs=3) as pt,
            tc.tile_pool(name="otn", bufs=2) as otn_pool,
            tc.tile_pool(name="yp", bufs=3) as yp,
            tc.tile_pool(name="sm", bufs=4) as sm,
            tc.tile_pool(name="psW", bufs=2, space="PSUM") as psW,
            tc.tile_pool(name="psOT", bufs=2, space="PSUM") as psOT,
            tc.tile_pool(name="ps2", bufs=2, space="PSUM") as ps2,
        ):
            # ---- persistent tensors ----
            qt_sb = pers.tile([P, BS], CD, tag="QT")
            kt_sb = pers.tile([P, BS], CD, tag="KT")
            v_sb = pers.tile([P, BS // P, 2 * HD + 2], CD, tag="V")
            wq_sb = pers.tile([P, KCH, CPC], CD, tag="wq")
            wk_sb = pers.tile([P, KCH, CPC], CD, tag="wk")
            wv_sb = pers.tile([P, KCH, CPC], CD, tag="wv")
            wo_sb = pers.tile([P, D], CD, tag="wo")
            bq_sb = pers.tile([1, CPC], CD, tag="bq")
            bk_sb = pers.tile([1, CPC], CD, tag="bk")
            bv_sb = pers.tile([1, CPC], CD, tag="bv")
            ones_sb = pers.tile([1, QT_TILE], CD, tag="ones")
            onesf_sb = pers.tile([1, HD], F32, tag="onesf")
            ident_sb = pers.tile([P, P], CD, tag="ident")

            nc.sync.dma_start(wq_sb[:], wq.rearrange("(o p) c -> p o c", p=P))
            nc.sync.dma_start(wk_sb[:], wk.rearrange("(o p) c -> p o c", p=P))
            nc.sync.dma_start(wv_sb[:], wv.rearrange("(o p) c -> p o c", p=P))
            nc.sync.dma_start(wo_sb[:], wo[:, :])
            nc.sync.dma_start(bq_sb[:], bq[:, :])
            nc.sync.dma_start(bk_sb[:], bk[:, :])
            nc.sync.dma_start(bv_sb[:], bv[:, :])
            nc.vector.memset(ones_sb[:], 1.0)
            nc.vector.memset(onesf_sb[:], 1.0)
            make_identity(nc, ident_sb[:])

            # ---- phase 1: projections, as a lazily-driven generator ----
            # Units are pulled from inside the attention loop so projection
            # matmuls (pure PE) fill the PE idle left by ACT-paced attention.
            XQ = BS // 4  # 2048 rows per x chunk

            def proj_gen():
                for xq in range(4):
                    xt = xin.tile([P, KCH, XQ], CD, tag="xt")
                    nc.sync.dma_start(
                        xt[:],
                        xT[:, xq * XQ : (xq + 1) * XQ].rearrange(
                            "(o p) q -> p o q", p=P
                        ),
                    )
                    yield
                    for lq in range(XQ // QT_TILE):
                        q0 = xq * XQ + lq * QT_TILE
                        l0 = lq * QT_TILE
                        for w_sb, b_sb, dst in (
                            (wq_sb, bq_sb, qt_sb),
                            (wk_sb, bk_sb, kt_sb),
                            (wv_sb, bv_sb, None),
                        ):
                            pj = psW.tile([P, QT_TILE], F32, tag="w", name="pj")
                            nc.tensor.matmul(pj[:], b_sb[0:1, :], ones_sb[0:1, :],
                                             start=True, stop=False)
                            yield
                            for o in range(KCH):
                                nc.tensor.matmul(
                                    pj[:], w_sb[:, o, :], xt[:, o, l0 : l0 + QT_TILE],
                                    start=False, stop=(o == KCH - 1),
                                )
                                yield
                            if dst is not None:
                                nc.vector.tensor_copy(
                                    out=dst[:, q0 : q0 + QT_TILE], in_=pj[:]
                                )
                                yield
                            else:
                                # V: VT chunk -> PE-transpose into natural
                                # layout with per-head ones-columns.
                                vt_sb = vtp.tile([P, QT_TILE], CD, tag="vt")
                                nc.vector.tensor_copy(out=vt_sb[:], in_=pj[:])
                                yield
                                for rt in range(QT_TILE // P):
                                    tp = psW.tile([P, P], CD, tag="w", name="tp")
                                    nc.tensor.transpose(
                                        tp[:], vt_sb[:, rt * P : (rt + 1) * P],
                                        ident_sb[:],
                                    )
                                    grt = q0 // P + rt
                                    nc.vector.tensor_copy(
                                        out=v_sb[:, grt, 0:HD], in_=tp[:, 0:HD]
                                    )
                                    nc.vector.tensor_copy(
                                        out=v_sb[:, grt, HD + 1 : 2 * HD + 1],
                                        in_=tp[:, HD:CPC],
                                    )
                                    nc.vector.memset(v_sb[:, grt, HD : HD + 1], 1.0)
                                    nc.vector.memset(
                                        v_sb[:, grt, 2 * HD + 1 : 2 * HD + 2], 1.0
                                    )
                                    yield

            gen = proj_gen()

            def pull(n):
                for _ in range(n):
                    if next(gen, "done") == "done":
                        break

            UNITS_PER_CHUNK = 1 + 4 * (10 + 10 + 14)
            pull(UNITS_PER_CHUNK)  # batch 0's projections up front

            # ---- phase 2+3: attention + output projection ----
            def emit_st_exp(b, qa, kt):
                q0 = b * S + qa * QT_TILE
                k0 = b * S + kt * P
                stp = ps2.tile([P, 2 * QT_TILE], F32, tag="stp", name="stp")
                for h in range(2):
                    hp = h * HD
                    nc.tensor.matmul(
                        stp[:, h * QT_TILE : (h + 1) * QT_TILE],
                        kt_sb[hp : hp + HD, k0 : k0 + P],
                        qt_sb[hp : hp + HD, q0 : q0 + QT_TILE],
                        start=True, stop=True,
                    )
                p_t = pt.tile([P, 2 * QT_TILE], CD, tag="p", name="p")
                nc.scalar.activation(
                    p_t[:], stp[:], mybir.ActivationFunctionType.Exp, scale=scale
                )
                return p_t

            def emit_av(ot, b, kt, p_t):
                krow = b * NKT + kt
                for h in range(2):
                    vcol = h * (HD + 1)
                    nc.tensor.matmul(
                        ot[h][0 : HD + 1, :],
                        v_sb[:, krow, vcol : vcol + HD + 1],
                        p_t[:, h * QT_TILE : (h + 1) * QT_TILE],
                        start=(kt == 0), stop=(kt == NKT - 1),
                    )

            def finalize(fin):
                b, qa, ot = fin
                q0 = b * S + qa * QT_TILE
                # softmax denominators live in row 64 of each ot tile
                sums = sm.tile([1, 2 * QT_TILE], F32, tag="sums")
                nc.vector.tensor_copy(
                    out=sums[0:1, 0:QT_TILE], in_=ot[0][HD : HD + 1, :]
                )
                nc.vector.tensor_copy(
                    out=sums[0:1, QT_TILE:], in_=ot[1][HD : HD + 1, :]
                )
                rps = psW.tile([P, QT_TILE], F32, tag="w", name="rps")
                for h in range(2):
                    nc.tensor.matmul(
                        rps[h * HD : (h + 1) * HD, :],
                        onesf_sb[0:1, :],
                        sums[0:1, h * QT_TILE : (h + 1) * QT_TILE],
                        start=True, stop=True,
                    )
                r_sb = sm.tile([P, QT_TILE], F32, tag="rsb")
                nc.vector.reciprocal(r_sb[:], rps[:])
                on = otn_pool.tile([P, QT_TILE], CD, tag="otn")
                nc.vector.tensor_mul(
                    out=on[0:HD, :], in0=ot[0][0:HD, :], in1=r_sb[0:HD, :]
                )
                nc.vector.tensor_mul(
                    out=on[HD:CPC, :], in0=ot[1][0:HD, :], in1=r_sb[HD:CPC, :]
                )
                # output projection: y[q0:q0+512, :] partial = on.T @ wo
                for j in range(QT_TILE // P):
                    ysb = yp.tile([P, D], F32, tag="y")
                    for e in range(D // QT_TILE):
                        yps = psOT.tile([P, QT_TILE], F32, tag="ot", name="yps")
                        nc.tensor.matmul(
                            yps[:],
                            on[:, j * P : (j + 1) * P],
                            wo_sb[:, e * QT_TILE : (e + 1) * QT_TILE],
                            start=True, stop=True,
                        )
                        nc.vector.tensor_copy(
                            out=ysb[:, e * QT_TILE : (e + 1) * QT_TILE], in_=yps[:]
                        )
                    nc.sync.dma_start(y[q0 + j * P : q0 + (j + 1) * P, :], ysb[:])

            blocks = [(b, qa) for b in range(B) for qa in range(NQA)]
            prev_fin = None
            for b, qa in blocks:
                pts = {0: emit_st_exp(b, qa, 0), 1: emit_st_exp(b, qa, 1)}
                if prev_fin is not None:
                    finalize(prev_fin)
                    prev_fin = None
                ot = [
                    psOT.tile([P, QT_TILE], F32, tag="ot", name=f"ot{h}")
                    for h in range(2)
                ]
                for kt in range(2, NKT):
                    pts[kt] = emit_st_exp(b, qa, kt)
                    emit_av(ot, b, kt - 2, pts.pop(kt - 2))
                    pull(3)
                emit_av(ot, b, NKT - 2, pts.pop(NKT - 2))
                emit_av(ot, b, NKT - 1, pts.pop(NKT - 1))
                prev_fin = (b, qa, ot)
            finalize(prev_fin)

    nc.compile()
    _NC_CACHE["nc"] = nc
    return nc


def make_in_maps(inputs):
    x = np.asarray(inputs["x"], np.float32)
    Wq = np.asarray(inputs["Wq"], np.float32)
    Wk = np.asarray(inputs["Wk"], np.float32)
    Wv = np.asarray(inputs["Wv"], np.float32)
    Wo = np.asarray(inputs["Wo"], np.float32)
    bq = np.asarray(inputs["bq"], np.float32)
    bk = np.asarray(inputs["bk"], np.float32)
    bv = np.asarray(inputs["bv"], np.float32)

    xT = np.ascontiguousarray(x.reshape(BS, D).T).astype(CD_NP)
    in_maps = []
    for c in range(NCORES):
        sl = slice(c * CPC, (c + 1) * CPC)
        in_maps.append(
            {
                "xT": xT,
                "wq": np.ascontiguousarray(Wq[:, sl]).astype(CD_NP),
                "wk": np.ascontiguousarray(Wk[:, sl]).astype(CD_NP),
                "wv": np.ascontiguousarray(Wv[:, sl]).astype(CD_NP),
                "wo": np.ascontiguousarray(Wo[sl, :]).astype(CD_NP),
                "bq": np.ascontiguousarray(bq[sl].reshape(1, CPC)).astype(CD_NP),
                "bk": np.ascontiguousarray(bk[sl].reshape(1, CPC)).astype(CD_NP),
                "bv": np.ascontiguousarray(bv[sl].reshape(1, CPC)).astype(CD_NP),
            }
        )
    return in_maps


def kernel(**inputs):
    global LAST_RESULTS
    bo = np.asarray(inputs["bo"], np.float32)
    nc = build_nc()
    in_maps = make_in_maps(inputs)
    res = bass_utils.run_bass_kernel_spmd(nc, in_maps, core_ids=list(range(NCORES)))
    LAST_RESULTS = res
    acc = np.zeros((BS, D), np.float64)
    for r in res.results:
        acc += r["y"].astype(np.float64)
    out = (acc + bo.astype(np.float64)).astype(np.float32)
    return out.reshape(B, S, D)


# revision 14
# speedup vs baseline: 1.6785x; 1.0017x over previous
"""Multi-head attention (B=4, S=2048, D=1024, H=16, Hd=64) on 8 TRN2 NeuronCores.

Sharding: tensor-parallel over heads — 2 heads per core (128 channels).
Each core computes its heads' Q/K/V projections, attention, and the partial
output projection (its 128 rows of Wo); the host sums the 8 partials + bo.

Device-side structure (per core):
  - x is pre-transposed on host to xT [D, B*S]; streamed in 4 big chunks.
  - Q, K produced transposed: QT/KT [128ch, B*S], heads stacked on
    partitions (head0 rows 0:64, head1 rows 64:128). The two heads' K=64
    score matmuls are emitted adjacently at disjoint row groups, so they
    run concurrently in the 128x128 PE array.
  - V is computed transposed (VT, N=512 matmuls) then PE-transposed into
    natural [seq, ch] layout with a ones-column per head; the attention
    output matmul OT[65, q] = V_aug.T @ P carries the softmax denominator
    in row 64 for free.
  - Both heads' score tiles share one 2-bank PSUM tile, so exp() runs as
    a single 1024-wide ACT op (half the ACT instruction count).
  - Attention is software-pipelined: AV matmuls lag the score matmuls by
    2 k-steps, and the normalization + output projection of block i is
    emitted inside block i+1's first score matmuls, so the PE never
    stalls on the ACT/DVE chains.
  - No max-subtraction in softmax: scores ~ N(0,1) by construction
    (|score| < ~7), exp() is safe in fp32.
"""
import sys

sys.path.insert(0, "/opt/trn_rl_repo")

import numpy as np
import ml_dtypes

import concourse.bass as bass
import concourse.mybir as mybir
import concourse.tile as tile
from concourse import bacc, bass_utils
from concourse.masks import make_identity

B, S, D = 4, 2048, 1024
BS = B * S            # 8192 rows
NCORES = 8
CPC = 128             # channels per core (2 heads x 64)
HD = 64               # head dim
P = 128
QT_TILE = 512         # q-tile width
NQT = BS // QT_TILE   # 16
NKT = S // P          # 16 k-tiles per batch
NQA = S // QT_TILE    # 4 q-tiles per batch

F32 = mybir.dt.float32
CD = mybir.dt.bfloat16          # compute dtype on device
CD_NP = ml_dtypes.bfloat16

LAST_RESULTS = None
_NC_CACHE = {}


def build_nc():
    if "nc" in _NC_CACHE:
        return _NC_CACHE["nc"]
    nc = bacc.Bacc(trn_type="TRN2", num_devices=NCORES)

    xT = nc.dram_tensor("xT", [D, BS], CD, kind="ExternalInput").ap()
    wq = nc.dram_tensor("wq", [D, CPC], CD, kind="ExternalInput").ap()
    wk = nc.dram_tensor("wk", [D, CPC], CD, kind="ExternalInput").ap()
    wv = nc.dram_tensor("wv", [D, CPC], CD, kind="ExternalInput").ap()
    wo = nc.dram_tensor("wo", [CPC, D], CD, kind="ExternalInput").ap()
    bq = nc.dram_tensor("bq", [1, CPC], CD, kind="ExternalInput").ap()
    bk = nc.dram_tensor("bk", [1, CPC], CD, kind="ExternalInput").ap()
    bv = nc.dram_tensor("bv", [1, CPC], CD, kind="ExternalInput").ap()
    y = nc.dram_tensor("y", [BS, D], F32, kind="ExternalOutput").ap()

    KCH = D // P  # 8 contraction chunks for the projections
    scale = float(1.0 / np.sqrt(np.float32(HD)))

    with tile.TileContext(nc) as tc:
        with (
            tc.tile_pool(name="pers", bufs=1) as pers,
            tc.tile_pool(name="xin", bufs=2) as xin,
            tc.tile_pool(name="vtp", bufs=2) as vtp,
            tc.tile_pool(name="pt", bufs=3) as pt,
            tc.tile_pool(name="otn", bufs=2) as otn_pool,
            tc.tile_pool(name="yp", bufs=3) as yp,
            tc.tile_pool(name="sm", bufs=4) as sm,
            tc.tile_pool(name="psW", bufs=2, space="PSUM") as psW,
            tc.tile_pool(name="psOT", bufs=2, space="PSUM") as psOT,
            tc.tile_pool(name="ps2", bufs=2, space="PSUM") as ps2,
        ):
            # ---- persistent tensors ----
            qt_sb = pers.tile([P, BS], CD, tag="QT")
            kt_sb = pers.tile([P, BS], CD, tag="KT")
            v_sb = pers.tile([P, BS // P, 2 * HD + 2], CD, tag="V")
            wq_sb = pers.tile([P, KCH, CPC], CD, tag="wq")
            wk_sb = pers.tile([P, KCH, CPC], CD, tag="wk")
            wv_sb = pers.tile([P, KCH, CPC], CD, tag="wv")
            wo_sb = pers.tile([P, D], CD, tag="wo")
            bq_sb = pers.tile([1, CPC], CD, tag="bq")
            bk_sb = pers.tile([1, CPC], CD, tag="bk")
            bv_sb = pers.tile([1, CPC], CD, tag="bv")
            ones_sb = pers.tile([1, QT_TILE], CD, tag="ones")
            onesf_sb = pers.tile([1, HD], F32, tag="onesf")
            ident_sb = pers.tile([P, P], CD, tag="ident")

            nc.sync.dma_start(wq_sb[:], wq.rearrange("(o p) c -> p o c", p=P))
            nc.sync.dma_start(wk_sb[:], wk.rearrange("(o p) c -> p o c", p=P))
            nc.sync.dma_start(wv_sb[:], wv.rearrange("(o p) c -> p o c", p=P))
            nc.sync.dma_start(wo_sb[:], wo[:, :])
            nc.sync.dma_start(bq_sb[:], bq[:, :])
            nc.sync.dma_start(bk_sb[:], bk[:, :])
            nc.sync.dma_start(bv_sb[:], bv[:, :])
            nc.vector.memset(ones_sb[:], 1.0)
            nc.vector.memset(onesf_sb[:], 1.0)
            make_identity(nc, ident_sb[:])

            # ---- phase 1: projections, as a lazily-driven generator ----
            # Units are pulled from inside the attention loop so projection
            # matmuls (pure PE) fill the PE idle left by ACT-paced attention.
            XQ = BS // 4  # 2048 rows per x chunk

            def proj_gen():
                for xq in range(4):
                    xt = xin.tile([P, KCH, XQ], CD, tag="xt")
                    nc.sync.dma_start(
                        xt[:],
                        xT[:, xq * XQ : (xq + 1) * XQ].rearrange(
                            "(o p) q -> p o q", p=P
                        ),
                    )
                    yield
                    for lq in range(XQ // QT_TILE):
                        q0 = xq * XQ + lq * QT_TILE
                        l0 = lq * QT_TILE
                        for w_sb, b_sb, dst in (
                            (wq_sb, bq_sb, qt_sb),
                            (wk_sb, bk_sb, kt_sb),
                            (wv_sb, bv_sb, None),
                        ):
                            pj = psW.tile([P, QT_TILE], F32, tag="w", name="pj")
                            nc.tensor.matmul(pj[:], b_sb[0:1, :], ones_sb[0:1, :],
                                             start=True, stop=False)
                            yield
                            for o in range(KCH):
                                nc.tensor.matmul(
                                    pj[:], w_sb[:, o, :], xt[:, o, l0 : l0 + QT_TILE],
                                    start=False, stop=(o == KCH - 1),
                                )
                                yield
                            if dst is not None:
                                nc.vector.tensor_copy(
                                    out=dst[:, q0 : q0 + QT_TILE], in_=pj[:]
                                )
                                yield
                            else:
                                # V: VT chunk -> PE-transpose into natural
                                # layout with per-head ones-columns.
                                vt_sb = vtp.tile([P, QT_TILE], CD, tag="vt")
                                nc.vector.tensor_copy(out=vt_sb[:], in_=pj[:])
                                yield
                                for rt in range(QT_TILE // P):
                                    tp = psW.tile([P, P], CD, tag="w", name="tp")
                                    nc.tensor.transpose(
                                        tp[:], vt_sb[:, rt * P : (rt + 1) * P],
                                        ident_sb[:],
                                    )
                                    grt = q0 // P + rt
                                    nc.vector.tensor_copy(
                                        out=v_sb[:, grt, 0:HD], in_=tp[:, 0:HD]
                                    )
                                    nc.vector.tensor_copy(
                                        out=v_sb[:, grt, HD + 1 : 2 * HD + 1],
                                        in_=tp[:, HD:CPC],
                                    )
                                    nc.vector.memset(v_sb[:, grt, HD : HD + 1], 1.0)
                                    nc.vector.memset(
                                        v_sb[:, grt, 2 * HD + 1 : 2 * HD + 2], 1.0
                                    )
                                    yield

            gen = proj_gen()

            def pull(n):
                for _ in range(n):
                    if next(gen, "done") == "done":
                        break

            UNITS_PER_CHUNK = 1 + 4 * (10 + 10 + 14)
            pull(UNITS_PER_CHUNK)  # batch 0's projections up front

            # ---- phase 2+3: attention + output projection ----
            def emit_st_exp(b, qa, kt):
                q0 = b * S + qa * QT_TILE
                k0 = b * S + kt * P
                stp = ps2.tile([P, 2 * QT_TILE], F32, tag="stp", name="stp")
                for h in range(2):
                    hp = h * HD
                    nc.tensor.matmul(
                        stp[:, h * QT_TILE : (h + 1) * QT_TILE],
                        kt_sb[hp : hp + HD, k0 : k0 + P],
                        qt_sb[hp : hp + HD, q0 : q0 + QT_TILE],
                        start=True, stop=True,
                    )
                p_t = pt.tile([P, 2 * QT_TILE], CD, tag="p", name="p")
                nc.scalar.activation(
                    p_t[:], stp[:], mybir.ActivationFunctionType.Exp, scale=scale
                )
                return p_t

            def emit_av(ot, b, kt, p_t):
                krow = b * NKT + kt
                for h in range(2):
                    vcol = h * (HD + 1)
                    nc.tensor.matmul(
                        ot[h][0 : HD + 1, :],
                        v_sb[:, krow, vcol : vcol + HD + 1],
                        p_t[:, h * QT_TILE : (h + 1) * QT_TILE],
                        start=(kt == 0), stop=(kt == NKT - 1),
                    )

            def finalize(fin):
                b, qa, ot = fin
                q0 = b * S + qa * QT_TILE
                # softmax denominators live in row 64 of each ot tile
                sums = sm.tile([1, 2 * QT_TILE], F32, tag="sums")
                nc.vector.tensor_copy(
                    out=sums[0:1, 0:QT_TILE], in_=ot[0][HD : HD + 1, :]
                )
                nc.vector.tensor_copy(
                    out=sums[0:1, QT_TILE:], in_=ot[1][HD : HD + 1, :]
                )
                rps = psW.tile([P, QT_TILE], F32, tag="w", name="rps")
                for h in range(2):
                    nc.tensor.matmul(
                        rps[h * HD : (h + 1) * HD, :],
                        onesf_sb[0:1, :],
                        sums[0:1, h * QT_TILE : (h + 1) * QT_TILE],
                        start=True, stop=True,
                    )
                r_sb = sm.tile([P, QT_TILE], F32, tag="rsb")
                nc.vector.reciprocal(r_sb[:], rps[:])
                on = otn_pool.tile([P, QT_TILE], CD, tag="otn")
                nc.vector.tensor_mul(
                    out=on[0:HD, :], in0=ot[0][0:HD, :], in1=r_sb[0:HD, :]
                )
                nc.vector.tensor_mul(
                    out=on[HD:CPC, :], in0=ot[1][0:HD, :], in1=r_sb[HD:CPC, :]
                )
                # output projection: y[q0:q0+512, :] partial = on.T @ wo
                for j in range(QT_TILE // P):
                    ysb = yp.tile([P, D], F32, tag="y")
                    for e in range(D // QT_TILE):
                        yps = psOT.tile([P, QT_TILE], F32, tag="ot", name="yps")
                        nc.tensor.matmul(
                            yps[:],
                            on[:, j * P : (j + 1) * P],
                            wo_sb[:, e * QT_TILE : (e + 1) * QT_TILE],
                            start=True, stop=True,
                        )
                        nc.vector.tensor_copy(
                            out=ysb[:, e * QT_TILE : (e + 1) * QT_TILE], in_=yps[:]
                        )
                    nc.sync.dma_start(y[q0 + j * P : q0 + (j + 1) * P, :], ysb[:])

            blocks = [(b, qa) for b in range(B) for qa in range(NQA)]
            prev_fin = None
            for b, qa in blocks:
                pts = {0: emit_st_exp(b, qa, 0), 1: emit_st_exp(b, qa, 1)}
                if prev_fin is not None:
                    finalize(prev_fin)
                    prev_fin = None
                ot = [
                    psOT.tile([P, QT_TILE], F32, tag="ot", name=f"ot{h}")
                    for h in range(2)
                ]
                for kt in range(2, NKT):
                    pts[kt] = emit_st_exp(b, qa, kt)
                    pull(3)
                    emit_av(ot, b, kt - 2, pts.pop(kt - 2))
                emit_av(ot, b, NKT - 2, pts.pop(NKT - 2))
                emit_av(ot, b, NKT - 1, pts.pop(NKT - 1))
                prev_fin = (b, qa, ot)
            finalize(prev_fin)

    nc.compile()
    _NC_CACHE["nc"] = nc
    return nc


def make_in_maps(inputs):
    x = np.asarray(inputs["x"], np.float32)
    Wq = np.asarray(inputs["Wq"], np.float32)
    Wk = np.asarray(inputs["Wk"], np.float32)
    Wv = np.asarray(inputs["Wv"], np.float32)
    Wo = np.asarray(inputs["Wo"], np.float32)
    bq = np.asarray(inputs["bq"], np.float32)
    bk = np.asarray(inputs["bk"], np.float32)
    bv = np.asarray(inputs["bv"], np.float32)

    xT = np.ascontiguousarray(x.reshape(BS, D).T).astype(CD_NP)
    in_maps = []
    for c in range(NCORES):
        sl = slice(c * CPC, (c + 1) * CPC)
        in_maps.append(
            {
                "xT": xT,
                "wq": np.ascontiguousarray(Wq[:, sl]).astype(CD_NP),
                "wk": np.ascontiguousarray(Wk[:, sl]).astype(CD_NP),
                "wv": np.ascontiguousarray(Wv[:, sl]).astype(CD_NP),
                "wo": np.ascontiguousarray(Wo[sl, :]).astype(CD_NP),
                "bq": np.ascontiguousarray(bq[sl].reshape(1, CPC)).astype(CD_NP),
                "bk": np.ascontiguousarray(bk[sl].reshape(1, CPC)).astype(CD_NP),
                "bv": np.ascontiguousarray(bv[sl].reshape(1, CPC)).astype(CD_NP),
            }
        )
    return in_maps


def kernel(**inputs):
    global LAST_RESULTS
    bo = np.asarray(inputs["bo"], np.float32)
    nc = build_nc()
    in_maps = make_in_maps(inputs)
    res = bass_utils.run_bass_kernel_spmd(nc, in_maps, core_ids=list(range(NCORES)))
    LAST_RESULTS = res
    acc = np.zeros((BS, D), np.float64)
    for r in res.results:
        acc += r["y"].astype(np.float64)
    out = (acc + bo.astype(np.float64)).astype(np.float32)
    return out.reshape(B, S, D)


# revision 17
# speedup vs baseline: 1.6981x; 1.0117x over previous
"""Multi-head attention (B=4, S=2048, D=1024, H=16, Hd=64) on 8 TRN2 NeuronCores.

Sharding: tensor-parallel over heads — 2 heads per core (128 channels).
Each core computes its heads' Q/K/V projections, attention, and the partial
output projection (its 128 rows of Wo); the host sums the 8 partials + bo.

Device-side structure (per core):
  - x is pre-transposed on host to xT [D, B*S]; streamed in 4 big chunks.
  - Q, K produced transposed: QT/KT [128ch, B*S], heads stacked on
    partitions (head0 rows 0:64, head1 rows 64:128). The two heads' K=64
    score matmuls are emitted adjacently at disjoint row groups, so they
    run concurrently in the 128x128 PE array.
  - V is computed transposed (VT, N=512 matmuls) then PE-transposed into
    natural [seq, ch] layout with a ones-column per head; the attention
    output matmul OT[65, q] = V_aug.T @ P carries the softmax denominator
    in row 64 for free.
  - Both heads' score tiles share one 2-bank PSUM tile, so exp() runs as
    a single 1024-wide ACT op (half the ACT instruction count).
  - Attention is software-pipelined: AV matmuls lag the score matmuls by
    2 k-steps, and the normalization + output projection of block i is
    emitted inside block i+1's first score matmuls, so the PE never
    stalls on the ACT/DVE chains.
  - No max-subtraction in softmax: scores ~ N(0,1) by construction
    (|score| < ~7), exp() is safe in fp32.
"""
import sys

sys.path.insert(0, "/opt/trn_rl_repo")

import numpy as np
import ml_dtypes

import concourse.bass as bass
import concourse.mybir as mybir
import concourse.tile as tile
from concourse import bacc, bass_utils
from concourse.masks import make_identity

B, S, D = 4, 2048, 1024
BS = B * S            # 8192 rows
NCORES = 8
CPC = 128             # channels per core (2 heads x 64)
HD = 64               # head dim
P = 128
QT_TILE = 512         # q-tile width
NQT = BS // QT_TILE   # 16
NKT = S // P          # 16 k-tiles per batch
NQA = S // QT_TILE    # 4 q-tiles per batch

F32 = mybir.dt.float32
CD = mybir.dt.bfloat16          # compute dtype on device
CD_NP = ml_dtypes.bfloat16

LAST_RESULTS = None
_NC_CACHE = {}


def build_nc():
    if "nc" in _NC_CACHE:
        return _NC_CACHE["nc"]
    nc = bacc.Bacc(trn_type="TRN2", num_devices=NCORES)

    xT = nc.dram_tensor("xT", [D, BS], CD, kind="ExternalInput").ap()
    wq = nc.dram_tensor("wq", [D, CPC], CD, kind="ExternalInput").ap()
    wk = nc.dram_tensor("wk", [D, CPC], CD, kind="ExternalInput").ap()
    wv = nc.dram_tensor("wv", [D, CPC], CD, kind="ExternalInput").ap()
    wo = nc.dram_tensor("wo", [CPC, D], CD, kind="ExternalInput").ap()
    bq = nc.dram_tensor("bq", [1, CPC], CD, kind="ExternalInput").ap()
    bk = nc.dram_tensor("bk", [1, CPC], CD, kind="ExternalInput").ap()
    bv = nc.dram_tensor("bv", [1, CPC], CD, kind="ExternalInput").ap()
    y = nc.dram_tensor("y", [BS, D], F32, kind="ExternalOutput").ap()

    KCH = D // P  # 8 contraction chunks for the projections
    scale = float(1.0 / np.sqrt(np.float32(HD)))

    with tile.TileContext(nc) as tc:
        with (
            tc.tile_pool(name="pers", bufs=1) as pers,
            tc.tile_pool(name="xin", bufs=2) as xin,
            tc.tile_pool(name="vtp", bufs=2) as vtp,
            tc.tile_pool(name="pt", bufs=4) as pt,
            tc.tile_pool(name="otn", bufs=2) as otn_pool,
            tc.tile_pool(name="yp", bufs=3) as yp,
            tc.tile_pool(name="sm", bufs=4) as sm,
            tc.tile_pool(name="psW", bufs=2, space="PSUM") as psW,
            tc.tile_pool(name="psOT", bufs=2, space="PSUM") as psOT,
            tc.tile_pool(name="ps2", bufs=2, space="PSUM") as ps2,
        ):
            # ---- persistent tensors ----
            qt_sb = pers.tile([P, BS], CD, tag="QT")
            kt_sb = pers.tile([P, BS], CD, tag="KT")
            v_sb = pers.tile([P, BS // P, 2 * HD + 2], CD, tag="V")
            wq_sb = pers.tile([P, KCH, CPC], CD, tag="wq")
            wk_sb = pers.tile([P, KCH, CPC], CD, tag="wk")
            wv_sb = pers.tile([P, KCH, CPC], CD, tag="wv")
            wo_sb = pers.tile([P, D], CD, tag="wo")
            bq_sb = pers.tile([1, CPC], CD, tag="bq")
            bk_sb = pers.tile([1, CPC], CD, tag="bk")
            bv_sb = pers.tile([1, CPC], CD, tag="bv")
            ones_sb = pers.tile([1, QT_TILE], CD, tag="ones")
            onesf_sb = pers.tile([1, HD], F32, tag="onesf")
            ident_sb = pers.tile([P, P], CD, tag="ident")

            nc.sync.dma_start(wq_sb[:], wq.rearrange("(o p) c -> p o c", p=P))
            nc.sync.dma_start(wk_sb[:], wk.rearrange("(o p) c -> p o c", p=P))
            nc.sync.dma_start(wv_sb[:], wv.rearrange("(o p) c -> p o c", p=P))
            nc.sync.dma_start(wo_sb[:], wo[:, :])
            nc.sync.dma_start(bq_sb[:], bq[:, :])
            nc.sync.dma_start(bk_sb[:], bk[:, :])
            nc.sync.dma_start(bv_sb[:], bv[:, :])
            nc.vector.memset(ones_sb[:], 1.0)
            nc.vector.memset(onesf_sb[:], 1.0)
            make_identity(nc, ident_sb[:])

            # ---- phase 1: projections, as a lazily-driven generator ----
            # Units are pulled from inside the attention loop so projection
            # matmuls (pure PE) fill the PE idle left by ACT-paced attention.
            XQ = BS // 4  # 2048 rows per x chunk

            def proj_gen():
                for xq in range(4):
                    xt = xin.tile([P, KCH, XQ], CD, tag="xt")
                    nc.sync.dma_start(
                        xt[:],
                        xT[:, xq * XQ : (xq + 1) * XQ].rearrange(
                            "(o p) q -> p o q", p=P
                        ),
                    )
                    yield
                    for lq in range(XQ // QT_TILE):
                        q0 = xq * XQ + lq * QT_TILE
                        l0 = lq * QT_TILE
                        for w_sb, b_sb, dst in (
                            (wq_sb, bq_sb, qt_sb),
                            (wk_sb, bk_sb, kt_sb),
                            (wv_sb, bv_sb, None),
                        ):
                            # one unit = one full 9-matmul accumulation run:
                            # consecutive same-bank matmuls stream at N/2.4;
                            # splitting a run across units breaks that.
                            pj = psW.tile([P, QT_TILE], F32, tag="w", name="pj")
                            nc.tensor.matmul(pj[:], b_sb[0:1, :], ones_sb[0:1, :],
                                             start=True, stop=False)
                            for o in range(KCH):
                                nc.tensor.matmul(
                                    pj[:], w_sb[:, o, :], xt[:, o, l0 : l0 + QT_TILE],
                                    start=False, stop=(o == KCH - 1),
                                )
                            if dst is not None:
                                nc.vector.tensor_copy(
                                    out=dst[:, q0 : q0 + QT_TILE], in_=pj[:]
                                )
                                yield
                            else:
                                # V: VT chunk -> PE-transpose into natural
                                # layout with per-head ones-columns.
                                vt_sb = vtp.tile([P, QT_TILE], CD, tag="vt")
                                nc.vector.tensor_copy(out=vt_sb[:], in_=pj[:])
                                yield
                                for rt in range(QT_TILE // P):
                                    tp = psW.tile([P, P], CD, tag="w", name="tp")
                                    nc.tensor.transpose(
                                        tp[:], vt_sb[:, rt * P : (rt + 1) * P],
                                        ident_sb[:],
                                    )
                                    grt = q0 // P + rt
                                    nc.vector.tensor_copy(
                                        out=v_sb[:, grt, 0:HD], in_=tp[:, 0:HD]
                                    )
                                    nc.vector.tensor_copy(
                                        out=v_sb[:, grt, HD + 1 : 2 * HD + 1],
                                        in_=tp[:, HD:CPC],
                                    )
                                    nc.vector.memset(v_sb[:, grt, HD : HD + 1], 1.0)
                                    nc.vector.memset(
                                        v_sb[:, grt, 2 * HD + 1 : 2 * HD + 2], 1.0
                                    )
                                yield

            gen = proj_gen()
            pulled = [0]

            def pull(n):
                for _ in range(n):
                    if next(gen, "done") == "done":
                        break
                    pulled[0] += 1

            UNITS_PER_CHUNK = 1 + 4 * 4
            pull(UNITS_PER_CHUNK)  # batch 0's projections up front

            # ---- phase 2+3: attention + output projection ----
            def emit_st_exp(b, qa, kt):
                q0 = b * S + qa * QT_TILE
                k0 = b * S + kt * P
                stp = ps2.tile([P, 2 * QT_TILE], F32, tag="stp", name="stp")
                for h in range(2):
                    hp = h * HD
                    nc.tensor.matmul(
                        stp[:, h * QT_TILE : (h + 1) * QT_TILE],
                        kt_sb[hp : hp + HD, k0 : k0 + P],
                        qt_sb[hp : hp + HD, q0 : q0 + QT_TILE],
                        start=True, stop=True,
                    )
                p_t = pt.tile([P, 2 * QT_TILE], CD, tag="p", name="p")
                nc.scalar.activation(
                    p_t[:], stp[:], mybir.ActivationFunctionType.Exp, scale=scale
                )
                return p_t

            def emit_av_group(ot, b, kts, ptd):
                # per head, run all kts back-to-back into the same OT bank
                # (same-bank accumulation streams on the PE)
                for h in range(2):
                    vcol = h * (HD + 1)
                    for kt in kts:
                        nc.tensor.matmul(
                            ot[h][0 : HD + 1, :],
                            v_sb[:, b * NKT + kt, vcol : vcol + HD + 1],
                            ptd[kt][:, h * QT_TILE : (h + 1) * QT_TILE],
                            start=(kt == 0), stop=(kt == NKT - 1),
                        )

            def finalize(fin):
                b, qa, ot = fin
                q0 = b * S + qa * QT_TILE
                # softmax denominators live in row 64 of each ot tile
                sums = sm.tile([1, 2 * QT_TILE], F32, tag="sums")
                nc.vector.tensor_copy(
                    out=sums[0:1, 0:QT_TILE], in_=ot[0][HD : HD + 1, :]
                )
                nc.vector.tensor_copy(
                    out=sums[0:1, QT_TILE:], in_=ot[1][HD : HD + 1, :]
                )
                rps = psW.tile([P, QT_TILE], F32, tag="w", name="rps")
                for h in range(2):
                    nc.tensor.matmul(
                        rps[h * HD : (h + 1) * HD, :],
                        onesf_sb[0:1, :],
                        sums[0:1, h * QT_TILE : (h + 1) * QT_TILE],
                        start=True, stop=True,
                    )
                r_sb = sm.tile([P, QT_TILE], F32, tag="rsb")
                nc.vector.reciprocal(r_sb[:], rps[:])
                on = otn_pool.tile([P, QT_TILE], CD, tag="otn")
                nc.vector.tensor_mul(
                    out=on[0:HD, :], in0=ot[0][0:HD, :], in1=r_sb[0:HD, :]
                )
                nc.vector.tensor_mul(
                    out=on[HD:CPC, :], in0=ot[1][0:HD, :], in1=r_sb[HD:CPC, :]
                )
                # output projection: y[q0:q0+512, :] partial = on.T @ wo
                for j in range(QT_TILE // P):
                    ysb = yp.tile([P, D], F32, tag="y")
                    for e in range(D // QT_TILE):
                        yps = psOT.tile([P, QT_TILE], F32, tag="ot", name="yps")
                        nc.tensor.matmul(
                            yps[:],
                            on[:, j * P : (j + 1) * P],
                            wo_sb[:, e * QT_TILE : (e + 1) * QT_TILE],
                            start=True, stop=True,
                        )
                        nc.vector.tensor_copy(
                            out=ysb[:, e * QT_TILE : (e + 1) * QT_TILE], in_=yps[:]
                        )
                    nc.sync.dma_start(y[q0 + j * P : q0 + (j + 1) * P, :], ysb[:])

            blocks = [(b, qa) for b in range(B) for qa in range(NQA)]
            prev_fin = None
            for bi, (b, qa) in enumerate(blocks):
                # all of batch b's projections must be emitted before its
                # attention reads them (deps are traced in emission order)
                deficit = UNITS_PER_CHUNK * (b + 1) - pulled[0]
                if deficit > 0:
                    pull(deficit)
                pts = {0: emit_st_exp(b, qa, 0), 1: emit_st_exp(b, qa, 1)}
                if prev_fin is not None:
                    finalize(prev_fin)
                    prev_fin = None
                ot = [
                    psOT.tile([P, QT_TILE], F32, tag="ot", name=f"ot{h}")
                    for h in range(2)
                ]
                for kt in range(2, NKT, 2):
                    pts[kt] = emit_st_exp(b, qa, kt)
                    pts[kt + 1] = emit_st_exp(b, qa, kt + 1)
                    if kt % 6 == 2:
                        pull(1)
                    emit_av_group(
                        ot, b, (kt - 2, kt - 1),
                        {kt - 2: pts.pop(kt - 2), kt - 1: pts.pop(kt - 1)},
                    )
                emit_av_group(
                    ot, b, (NKT - 2, NKT - 1),
                    {NKT - 2: pts.pop(NKT - 2), NKT - 1: pts.pop(NKT - 1)},
                )
                prev_fin = (b, qa, ot)
            finalize(prev_fin)

    nc.compile()
    _NC_CACHE["nc"] = nc
    return nc


def make_in_maps(inputs):
    x = np.asarray(inputs["x"], np.float32)
    Wq = np.asarray(inputs["Wq"], np.float32)
    Wk = np.asarray(inputs["Wk"], np.float32)
    Wv = np.asarray(inputs["Wv"], np.float32)
    Wo = np.asarray(inputs["Wo"], np.float32)
    bq = np.asarray(inputs["bq"], np.float32)
    bk = np.asarray(inputs["bk"], np.float32)
    bv = np.asarray(inputs["bv"], np.float32)

    xT = np.ascontiguousarray(x.reshape(BS, D).T).astype(CD_NP)
    in_maps = []
    for c in range(NCORES):
        sl = slice(c * CPC, (c + 1) * CPC)
        in_maps.append(
            {
                "xT": xT,
                "wq": np.ascontiguousarray(Wq[:, sl]).astype(CD_NP),
                "wk": np.ascontiguousarray(Wk[:, sl]).astype(CD_NP),
                "wv": np.ascontiguousarray(Wv[:, sl]).astype(CD_NP),
                "wo": np.ascontiguousarray(Wo[sl, :]).astype(CD_NP),
                "bq": np.ascontiguousarray(bq[sl].reshape(1, CPC)).astype(CD_NP),
                "bk": np.ascontiguousarray(bk[sl].reshape(1, CPC)).astype(CD_NP),
                "bv": np.ascontiguousarray(bv[sl].reshape(1, CPC)).astype(CD_NP),
            }
        )
    return in_maps


def kernel(**inputs):
    global LAST_RESULTS
    bo = np.asarray(inputs["bo"], np.float32)
    nc = build_nc()
    in_maps = make_in_maps(inputs)
    res = bass_utils.run_bass_kernel_spmd(nc, in_maps, core_ids=list(range(NCORES)))
    LAST_RESULTS = res
    acc = np.zeros((BS, D), np.float64)
    for r in res.results:
        acc += r["y"].astype(np.float64)
    out = (acc + bo.astype(np.float64)).astype(np.float32)
    return out.reshape(B, S, D)


# revision 18
# speedup vs baseline: 1.7575x; 1.0350x over previous
"""Multi-head attention (B=4, S=2048, D=1024, H=16, Hd=64) on 8 TRN2 NeuronCores.

Sharding: tensor-parallel over heads — 2 heads per core (128 channels).
Each core computes its heads' Q/K/V projections, attention, and the partial
output projection (its 128 rows of Wo); the host sums the 8 partials + bo.

Device-side structure (per core):
  - x is pre-transposed on host to xT [D, B*S]; streamed in 4 big chunks.
  - Q, K produced transposed: QT/KT [128ch, B*S], heads stacked on
    partitions (head0 rows 0:64, head1 rows 64:128). The two heads' K=64
    score matmuls are emitted adjacently at disjoint row groups, so they
    run concurrently in the 128x128 PE array.
  - V is computed transposed (VT, N=512 matmuls) then PE-transposed into
    natural [seq, ch] layout with a ones-column per head; the attention
    output matmul OT[65, q] = V_aug.T @ P carries the softmax denominator
    in row 64 for free.
  - Both heads' score tiles share one 2-bank PSUM tile, so exp() runs as
    a single 1024-wide ACT op (half the ACT instruction count).
  - Attention is software-pipelined: AV matmuls lag the score matmuls by
    2 k-steps, and the normalization + output projection of block i is
    emitted inside block i+1's first score matmuls, so the PE never
    stalls on the ACT/DVE chains.
  - No max-subtraction in softmax: scores ~ N(0,1) by construction
    (|score| < ~7), exp() is safe in fp32.
"""
import sys

sys.path.insert(0, "/opt/trn_rl_repo")

import numpy as np
import ml_dtypes

import concourse.bass as bass
import concourse.mybir as mybir
import concourse.tile as tile
from concourse import bacc, bass_utils
from concourse.masks import make_identity

B, S, D = 4, 2048, 1024
BS = B * S            # 8192 rows
NCORES = 8
CPC = 128             # channels per core (2 heads x 64)
HD = 64               # head dim
P = 128
QT_TILE = 512         # q-tile width
NQT = BS // QT_TILE   # 16
NKT = S // P          # 16 k-tiles per batch
NQA = S // QT_TILE    # 4 q-tiles per batch

F32 = mybir.dt.float32
CD = mybir.dt.bfloat16          # compute dtype on device
CD_NP = ml_dtypes.bfloat16

LAST_RESULTS = None
_NC_CACHE = {}


def build_nc():
    if "nc" in _NC_CACHE:
        return _NC_CACHE["nc"]
    nc = bacc.Bacc(trn_type="TRN2", num_devices=NCORES)

    xT = nc.dram_tensor("xT", [D, BS], CD, kind="ExternalInput").ap()
    wq = nc.dram_tensor("wq", [D, CPC], CD, kind="ExternalInput").ap()
    wk = nc.dram_tensor("wk", [D, CPC], CD, kind="ExternalInput").ap()
    wv = nc.dram_tensor("wv", [D, CPC], CD, kind="ExternalInput").ap()
    wo = nc.dram_tensor("wo", [CPC, D], CD, kind="ExternalInput").ap()
    bq = nc.dram_tensor("bq", [1, CPC], CD, kind="ExternalInput").ap()
    bk = nc.dram_tensor("bk", [1, CPC], CD, kind="ExternalInput").ap()
    bv = nc.dram_tensor("bv", [1, CPC], CD, kind="ExternalInput").ap()
    y = nc.dram_tensor("y", [BS, D], F32, kind="ExternalOutput").ap()

    KCH = D // P  # 8 contraction chunks for the projections
    scale = float(1.0 / np.sqrt(np.float32(HD)))

    with tile.TileContext(nc) as tc:
        with (
            tc.tile_pool(name="pers", bufs=1) as pers,
            tc.tile_pool(name="xin", bufs=2) as xin,
            tc.tile_pool(name="vtp", bufs=2) as vtp,
            tc.tile_pool(name="pt", bufs=4) as pt,
            tc.tile_pool(name="otn", bufs=2) as otn_pool,
            tc.tile_pool(name="yp", bufs=3) as yp,
            tc.tile_pool(name="sm", bufs=4) as sm,
            tc.tile_pool(name="psW", bufs=2, space="PSUM") as psW,
            tc.tile_pool(name="psOT", bufs=2, space="PSUM") as psOT,
            tc.tile_pool(name="ps2", bufs=2, space="PSUM") as ps2,
        ):
            # ---- persistent tensors ----
            qt_sb = pers.tile([P, BS], CD, tag="QT")
            kt_sb = pers.tile([P, BS], CD, tag="KT")
            v_sb = pers.tile([P, BS // P, 2 * HD + 2], CD, tag="V")
            wq_sb = pers.tile([P, KCH, CPC], CD, tag="wq")
            wk_sb = pers.tile([P, KCH, CPC], CD, tag="wk")
            wv_sb = pers.tile([P, KCH, CPC], CD, tag="wv")
            wo_sb = pers.tile([P, D], CD, tag="wo")
            bq_sb = pers.tile([1, CPC], CD, tag="bq")
            bk_sb = pers.tile([1, CPC], CD, tag="bk")
            bv_sb = pers.tile([1, CPC], CD, tag="bv")
            ones_sb = pers.tile([1, QT_TILE], CD, tag="ones")
            onesf_sb = pers.tile([1, HD], F32, tag="onesf")
            ident_sb = pers.tile([P, P], CD, tag="ident")

            nc.sync.dma_start(wq_sb[:], wq.rearrange("(o p) c -> p o c", p=P))
            nc.sync.dma_start(wk_sb[:], wk.rearrange("(o p) c -> p o c", p=P))
            nc.sync.dma_start(wv_sb[:], wv.rearrange("(o p) c -> p o c", p=P))
            nc.sync.dma_start(wo_sb[:], wo[:, :])
            nc.sync.dma_start(bq_sb[:], bq[:, :])
            nc.sync.dma_start(bk_sb[:], bk[:, :])
            nc.sync.dma_start(bv_sb[:], bv[:, :])
            nc.vector.memset(ones_sb[:], 1.0)
            nc.vector.memset(onesf_sb[:], 1.0)
            make_identity(nc, ident_sb[:])

            # ---- phase 1: projections, as a lazily-driven generator ----
            # Units are pulled from inside the attention loop so projection
            # matmuls (pure PE) fill the PE idle left by ACT-paced attention.
            XQ = BS // 4  # 2048 rows per x chunk

            def proj_gen():
                for xq in range(4):
                    xt = xin.tile([P, KCH, XQ], CD, tag="xt")
                    nc.sync.dma_start(
                        xt[:],
                        xT[:, xq * XQ : (xq + 1) * XQ].rearrange(
                            "(o p) q -> p o q", p=P
                        ),
                    )
                    yield
                    for lq in range(XQ // QT_TILE):
                        q0 = xq * XQ + lq * QT_TILE
                        l0 = lq * QT_TILE
                        for w_sb, b_sb, dst in (
                            (wq_sb, bq_sb, qt_sb),
                            (wk_sb, bk_sb, kt_sb),
                            (wv_sb, bv_sb, None),
                        ):
                            # one unit = one full 9-matmul accumulation run:
                            # consecutive same-bank matmuls stream at N/2.4;
                            # splitting a run across units breaks that.
                            pj = psW.tile([P, QT_TILE], F32, tag="w", name="pj")
                            nc.tensor.matmul(pj[:], b_sb[0:1, :], ones_sb[0:1, :],
                                             start=True, stop=False)
                            for o in range(KCH):
                                nc.tensor.matmul(
                                    pj[:], w_sb[:, o, :], xt[:, o, l0 : l0 + QT_TILE],
                                    start=False, stop=(o == KCH - 1),
                                )
                            if dst is not None:
                                nc.vector.tensor_copy(
                                    out=dst[:, q0 : q0 + QT_TILE], in_=pj[:]
                                )
                                yield
                            else:
                                # V: VT chunk -> PE-transpose into natural
                                # layout with per-head ones-columns.
                                vt_sb = vtp.tile([P, QT_TILE], CD, tag="vt")
                                nc.vector.tensor_copy(out=vt_sb[:], in_=pj[:])
                                yield
                                for rt in range(QT_TILE // P):
                                    tp = psW.tile([P, P], CD, tag="w", name="tp")
                                    nc.tensor.transpose(
                                        tp[:], vt_sb[:, rt * P : (rt + 1) * P],
                                        ident_sb[:],
                                    )
                                    grt = q0 // P + rt
                                    nc.vector.tensor_copy(
                                        out=v_sb[:, grt, 0:HD], in_=tp[:, 0:HD]
                                    )
                                    nc.vector.tensor_copy(
                                        out=v_sb[:, grt, HD + 1 : 2 * HD + 1],
                                        in_=tp[:, HD:CPC],
                                    )
                                    nc.vector.memset(v_sb[:, grt, HD : HD + 1], 1.0)
                                    nc.vector.memset(
                                        v_sb[:, grt, 2 * HD + 1 : 2 * HD + 2], 1.0
                                    )
                                yield

            gen = proj_gen()
            pulled = [0]

            def pull(n):
                for _ in range(n):
                    if next(gen, "done") == "done":
                        break
                    pulled[0] += 1

            UNITS_PER_CHUNK = 1 + 4 * 4
            pull(UNITS_PER_CHUNK)  # batch 0's projections up front

            # ---- phase 2+3: attention + output projection ----
            def emit_st_exp(b, qa, kt):
                q0 = b * S + qa * QT_TILE
                k0 = b * S + kt * P
                stp = ps2.tile([P, 2 * QT_TILE], F32, tag="stp", name="stp")
                for h in range(2):
                    hp = h * HD
                    nc.tensor.matmul(
                        stp[:, h * QT_TILE : (h + 1) * QT_TILE],
                        kt_sb[hp : hp + HD, k0 : k0 + P],
                        qt_sb[hp : hp + HD, q0 : q0 + QT_TILE],
                        start=True, stop=True,
                    )
                p_t = pt.tile([P, 2 * QT_TILE], CD, tag="p", name="p")
                nc.scalar.activation(
                    p_t[:], stp[:], mybir.ActivationFunctionType.Exp, scale=scale
                )
                return p_t

            def emit_av_group(ot, b, kts, ptd):
                # per head, run all kts back-to-back into the same OT bank
                # (same-bank accumulation streams on the PE)
                for h in range(2):
                    vcol = h * (HD + 1)
                    for kt in kts:
                        nc.tensor.matmul(
                            ot[h][0 : HD + 1, :],
                            v_sb[:, b * NKT + kt, vcol : vcol + HD + 1],
                            ptd[kt][:, h * QT_TILE : (h + 1) * QT_TILE],
                            start=(kt == 0), stop=(kt == NKT - 1),
                        )

            def finalize(fin):
                b, qa, ot = fin
                q0 = b * S + qa * QT_TILE
                # softmax denominators live in row 64 of each ot tile
                sums = sm.tile([1, 2 * QT_TILE], F32, tag="sums")
                nc.vector.tensor_copy(
                    out=sums[0:1, 0:QT_TILE], in_=ot[0][HD : HD + 1, :]
                )
                nc.vector.tensor_copy(
                    out=sums[0:1, QT_TILE:], in_=ot[1][HD : HD + 1, :]
                )
                pull(1)  # stream a projection run while the DVE chain runs
                rps = psW.tile([P, QT_TILE], F32, tag="w", name="rps")
                for h in range(2):
                    nc.tensor.matmul(
                        rps[h * HD : (h + 1) * HD, :],
                        onesf_sb[0:1, :],
                        sums[0:1, h * QT_TILE : (h + 1) * QT_TILE],
                        start=True, stop=True,
                    )
                r_sb = sm.tile([P, QT_TILE], F32, tag="rsb")
                nc.vector.reciprocal(r_sb[:], rps[:])
                on = otn_pool.tile([P, QT_TILE], CD, tag="otn")
                nc.vector.tensor_mul(
                    out=on[0:HD, :], in0=ot[0][0:HD, :], in1=r_sb[0:HD, :]
                )
                nc.vector.tensor_mul(
                    out=on[HD:CPC, :], in0=ot[1][0:HD, :], in1=r_sb[HD:CPC, :]
                )
                pull(1)
                # output projection: y[q0:q0+512, :] partial = on.T @ wo
                for j in range(QT_TILE // P):
                    ysb = yp.tile([P, D], F32, tag="y")
                    for e in range(D // QT_TILE):
                        yps = psOT.tile([P, QT_TILE], F32, tag="ot", name="yps")
                        nc.tensor.matmul(
                            yps[:],
                            on[:, j * P : (j + 1) * P],
                            wo_sb[:, e * QT_TILE : (e + 1) * QT_TILE],
                            start=True, stop=True,
                        )
                        nc.vector.tensor_copy(
                            out=ysb[:, e * QT_TILE : (e + 1) * QT_TILE], in_=yps[:]
                        )
                    nc.sync.dma_start(y[q0 + j * P : q0 + (j + 1) * P, :], ysb[:])

            blocks = [(b, qa) for b in range(B) for qa in range(NQA)]
            prev_fin = None
            for bi, (b, qa) in enumerate(blocks):
                # all of batch b's projections must be emitted before its
                # attention reads them (deps are traced in emission order)
                deficit = UNITS_PER_CHUNK * (b + 1) - pulled[0]
                if deficit > 0:
                    pull(deficit)
                pts = {0: emit_st_exp(b, qa, 0), 1: emit_st_exp(b, qa, 1)}
                if prev_fin is not None:
                    finalize(prev_fin)
                    prev_fin = None
                ot = [
                    psOT.tile([P, QT_TILE], F32, tag="ot", name=f"ot{h}")
                    for h in range(2)
                ]
                for kt in range(2, NKT, 2):
                    pts[kt] = emit_st_exp(b, qa, kt)
                    pts[kt + 1] = emit_st_exp(b, qa, kt + 1)
                    if kt % 6 == 2:
                        pull(1)
                    emit_av_group(
                        ot, b, (kt - 2, kt - 1),
                        {kt - 2: pts.pop(kt - 2), kt - 1: pts.pop(kt - 1)},
                    )
                emit_av_group(
                    ot, b, (NKT - 2, NKT - 1),
                    {NKT - 2: pts.pop(NKT - 2), NKT - 1: pts.pop(NKT - 1)},
                )
                prev_fin = (b, qa, ot)
            finalize(prev_fin)

    nc.compile()
    _NC_CACHE["nc"] = nc
    return nc


def make_in_maps(inputs):
    x = np.asarray(inputs["x"], np.float32)
    Wq = np.asarray(inputs["Wq"], np.float32)
    Wk = np.asarray(inputs["Wk"], np.float32)
    Wv = np.asarray(inputs["Wv"], np.float32)
    Wo = np.asarray(inputs["Wo"], np.float32)
    bq = np.asarray(inputs["bq"], np.float32)
    bk = np.asarray(inputs["bk"], np.float32)
    bv = np.asarray(inputs["bv"], np.float32)

    xT = np.ascontiguousarray(x.reshape(BS, D).T).astype(CD_NP)
    in_maps = []
    for c in range(NCORES):
        sl = slice(c * CPC, (c + 1) * CPC)
        in_maps.append(
            {
                "xT": xT,
                "wq": np.ascontiguousarray(Wq[:, sl]).astype(CD_NP),
                "wk": np.ascontiguousarray(Wk[:, sl]).astype(CD_NP),
                "wv": np.ascontiguousarray(Wv[:, sl]).astype(CD_NP),
                "wo": np.ascontiguousarray(Wo[sl, :]).astype(CD_NP),
                "bq": np.ascontiguousarray(bq[sl].reshape(1, CPC)).astype(CD_NP),
                "bk": np.ascontiguousarray(bk[sl].reshape(1, CPC)).astype(CD_NP),
                "bv": np.ascontiguousarray(bv[sl].reshape(1, CPC)).astype(CD_NP),
            }
        )
    return in_maps


def kernel(**inputs):
    global LAST_RESULTS
    bo = np.asarray(inputs["bo"], np.float32)
    nc = build_nc()
    in_maps = make_in_maps(inputs)
    res = bass_utils.run_bass_kernel_spmd(nc, in_maps, core_ids=list(range(NCORES)))
    LAST_RESULTS = res
    acc = np.zeros((BS, D), np.float64)
    for r in res.results:
        acc += r["y"].astype(np.float64)
    out = (acc + bo.astype(np.float64)).astype(np.float32)
    return out.reshape(B, S, D)


# revision 21
# speedup vs baseline: 1.9244x; 1.0950x over previous
"""Multi-head attention (B=4, S=2048, D=1024, H=16, Hd=64) on 8 TRN2 NeuronCores.

Sharding: tensor-parallel over heads — 2 heads per core (128 channels).
Each core computes its heads' Q/K/V projections, attention, and the partial
output projection (its 128 rows of Wo); the host sums the 8 partials + bo.

Device-side structure (per core):
  - x is pre-transposed on host to xT [D, B*S]; streamed in 4 big chunks.
  - Q, K produced transposed: QT/KT [128ch, B*S], heads stacked on
    partitions (head0 rows 0:64, head1 rows 64:128). The two heads' K=64
    score matmuls are emitted adjacently at disjoint row groups, so they
    run concurrently in the 128x128 PE array.
  - V is computed transposed (VT, N=512 matmuls) then PE-transposed into
    natural [seq, ch] layout with a ones-column per head; the attention
    output matmul OT[65, q] = V_aug.T @ P carries the softmax denominator
    in row 64 for free.
  - Both heads' score tiles share one 2-bank PSUM tile, so exp() runs as
    a single 1024-wide ACT op (half the ACT instruction count).
  - Attention is software-pipelined: AV matmuls lag the score matmuls by
    2 k-steps, and the normalization + output projection of block i is
    emitted inside block i+1's first score matmuls, so the PE never
    stalls on the ACT/DVE chains.
  - No max-subtraction in softmax: scores ~ N(0,1) by construction
    (|score| < ~7), exp() is safe in fp32.
"""
import sys

sys.path.insert(0, "/opt/trn_rl_repo")

import numpy as np
import ml_dtypes

import concourse.bass as bass
import concourse.mybir as mybir
import concourse.tile as tile
from concourse import bacc, bass_utils
from concourse.masks import make_identity

B, S, D = 4, 2048, 1024
BS = B * S            # 8192 rows
NCORES = 8
CPC = 128             # channels per core (2 heads x 64)
HD = 64               # head dim
P = 128
QT_TILE = 512         # q-tile width
NQT = BS // QT_TILE   # 16
NKT = S // P          # 16 k-tiles per batch
NQA = S // QT_TILE    # 4 q-tiles per batch

F32 = mybir.dt.float32
CD = mybir.dt.bfloat16          # compute dtype on device
CD_NP = ml_dtypes.bfloat16

LAST_RESULTS = None
_NC_CACHE = {}


def build_nc():
    if "nc" in _NC_CACHE:
        return _NC_CACHE["nc"]
    nc = bacc.Bacc(trn_type="TRN2", num_devices=NCORES)

    xT = nc.dram_tensor("xT", [D, BS], CD, kind="ExternalInput").ap()
    wq = nc.dram_tensor("wq", [D, CPC], CD, kind="ExternalInput").ap()
    wk = nc.dram_tensor("wk", [D, CPC], CD, kind="ExternalInput").ap()
    wv = nc.dram_tensor("wv", [D, CPC], CD, kind="ExternalInput").ap()
    wo = nc.dram_tensor("wo", [CPC, D], CD, kind="ExternalInput").ap()
    bq = nc.dram_tensor("bq", [1, CPC], CD, kind="ExternalInput").ap()
    bk = nc.dram_tensor("bk", [1, CPC], CD, kind="ExternalInput").ap()
    bv = nc.dram_tensor("bv", [1, CPC], CD, kind="ExternalInput").ap()
    y = nc.dram_tensor("y", [BS, D], F32, kind="ExternalOutput").ap()

    KCH = D // P  # 8 contraction chunks for the projections
    scale = float(1.0 / np.sqrt(np.float32(HD)))

    with tile.TileContext(nc) as tc:
        with (
            tc.tile_pool(name="pers", bufs=1) as pers,
            tc.tile_pool(name="xin", bufs=2) as xin,
            tc.tile_pool(name="vtp", bufs=2) as vtp,
            tc.tile_pool(name="pt", bufs=4) as pt,
            tc.tile_pool(name="otn", bufs=2) as otn_pool,
            tc.tile_pool(name="yp", bufs=3) as yp,
            tc.tile_pool(name="sm", bufs=4) as sm,
            tc.tile_pool(name="otu", bufs=4) as otu_pool,
            tc.tile_pool(name="psW", bufs=2, space="PSUM") as psW,
            tc.tile_pool(name="psOT", bufs=2, space="PSUM") as psOT,
            tc.tile_pool(name="ps2", bufs=2, space="PSUM") as ps2,
        ):
            # ---- persistent tensors ----
            qt_sb = pers.tile([P, BS], CD, tag="QT")
            kt_sb = pers.tile([P, BS], CD, tag="KT")
            v_sb = pers.tile([P, BS // P, 2 * HD + 2], CD, tag="V")
            wq_sb = pers.tile([P, KCH, CPC], CD, tag="wq")
            wk_sb = pers.tile([P, KCH, CPC], CD, tag="wk")
            wv_sb = pers.tile([P, KCH, CPC], CD, tag="wv")
            wo_sb = pers.tile([P, D], CD, tag="wo")
            bq_sb = pers.tile([1, CPC], CD, tag="bq")
            bk_sb = pers.tile([1, CPC], CD, tag="bk")
            bv_sb = pers.tile([1, CPC], CD, tag="bv")
            ones_sb = pers.tile([1, QT_TILE], CD, tag="ones")
            onesf_sb = pers.tile([P, HD], F32, tag="onesf")
            ident_sb = pers.tile([P, P], CD, tag="ident")

            nc.sync.dma_start(wq_sb[:], wq.rearrange("(o p) c -> p o c", p=P))
            nc.sync.dma_start(wk_sb[:], wk.rearrange("(o p) c -> p o c", p=P))
            nc.sync.dma_start(wv_sb[:], wv.rearrange("(o p) c -> p o c", p=P))
            nc.sync.dma_start(wo_sb[:], wo[:, :])
            nc.sync.dma_start(bq_sb[:], bq[:, :])
            nc.sync.dma_start(bk_sb[:], bk[:, :])
            nc.sync.dma_start(bv_sb[:], bv[:, :])
            nc.vector.memset(ones_sb[:], 1.0)
            nc.vector.memset(onesf_sb[:], 1.0)
            make_identity(nc, ident_sb[:])

            # ---- phase 1: projections, as a lazily-driven generator ----
            # Units are pulled from inside the attention loop so projection
            # matmuls (pure PE) fill the PE idle left by ACT-paced attention.
            XQ = BS // 4  # 2048 rows per x chunk

            def proj_gen():
                for xq in range(4):
                    xt = xin.tile([P, KCH, XQ], CD, tag="xt")
                    nc.sync.dma_start(
                        xt[:],
                        xT[:, xq * XQ : (xq + 1) * XQ].rearrange(
                            "(o p) q -> p o q", p=P
                        ),
                    )
                    yield
                    for lq in range(XQ // QT_TILE):
                        q0 = xq * XQ + lq * QT_TILE
                        l0 = lq * QT_TILE
                        for w_sb, b_sb, dst in (
                            (wq_sb, bq_sb, qt_sb),
                            (wk_sb, bk_sb, kt_sb),
                            (wv_sb, bv_sb, None),
                        ):
                            # one unit = one full 9-matmul accumulation run:
                            # consecutive same-bank matmuls stream at N/2.4;
                            # splitting a run across units breaks that.
                            pj = psW.tile([P, QT_TILE], F32, tag="w", name="pj")
                            nc.tensor.matmul(pj[:], b_sb[0:1, :], ones_sb[0:1, :],
                                             start=True, stop=False)
                            for o in range(KCH):
                                nc.tensor.matmul(
                                    pj[:], w_sb[:, o, :], xt[:, o, l0 : l0 + QT_TILE],
                                    start=False, stop=(o == KCH - 1),
                                )
                            if dst is not None:
                                nc.vector.tensor_copy(
                                    out=dst[:, q0 : q0 + QT_TILE], in_=pj[:]
                                )
                                yield
                            else:
                                # V: VT chunk -> PE-transpose into natural
                                # layout with per-head ones-columns.
                                vt_sb = vtp.tile([P, QT_TILE], CD, tag="vt")
                                nc.vector.tensor_copy(out=vt_sb[:], in_=pj[:])
                                yield
                                for rt in range(QT_TILE // P):
                                    tp = psW.tile([P, P], CD, tag="w", name="tp")
                                    nc.tensor.transpose(
                                        tp[:], vt_sb[:, rt * P : (rt + 1) * P],
                                        ident_sb[:],
                                    )
                                    grt = q0 // P + rt
                                    nc.vector.tensor_copy(
                                        out=v_sb[:, grt, 0:HD], in_=tp[:, 0:HD]
                                    )
                                    nc.vector.tensor_copy(
                                        out=v_sb[:, grt, HD + 1 : 2 * HD + 1],
                                        in_=tp[:, HD:CPC],
                                    )
                                    nc.vector.memset(v_sb[:, grt, HD : HD + 1], 1.0)
                                    nc.vector.memset(
                                        v_sb[:, grt, 2 * HD + 1 : 2 * HD + 2], 1.0
                                    )
                                yield

            gen = proj_gen()
            pulled = [0]

            def pull(n):
                for _ in range(n):
                    if next(gen, "done") == "done":
                        break
                    pulled[0] += 1

            UNITS_PER_CHUNK = 1 + 4 * 4
            pull(UNITS_PER_CHUNK)  # batch 0's projections up front

            # ---- phase 2+3: attention + output projection ----
            def emit_st_exp(b, qa, kt):
                q0 = b * S + qa * QT_TILE
                k0 = b * S + kt * P
                stp = ps2.tile([P, 2 * QT_TILE], F32, tag="stp", name="stp")
                for h in range(2):
                    hp = h * HD
                    nc.tensor.matmul(
                        stp[:, h * QT_TILE : (h + 1) * QT_TILE],
                        kt_sb[hp : hp + HD, k0 : k0 + P],
                        qt_sb[hp : hp + HD, q0 : q0 + QT_TILE],
                        start=True, stop=True,
                    )
                p_t = pt.tile([P, 2 * QT_TILE], CD, tag="p", name="p")
                nc.scalar.activation(
                    p_t[:], stp[:], mybir.ActivationFunctionType.Exp, scale=scale
                )
                return p_t

            def emit_av_group(ot, b, kts, ptd):
                # per head, run all kts back-to-back into the same OT bank
                # (same-bank accumulation streams on the PE)
                for h in range(2):
                    vcol = h * (HD + 1)
                    for kt in kts:
                        nc.tensor.matmul(
                            ot[h][0 : HD + 1, :],
                            v_sb[:, b * NKT + kt, vcol : vcol + HD + 1],
                            ptd[kt][:, h * QT_TILE : (h + 1) * QT_TILE],
                            start=(kt == 0), stop=(kt == NKT - 1),
                        )

            def finalize(fin):
                b, qa, otu = fin
                q0 = b * S + qa * QT_TILE
                # denominators live in row 64 of the evacuated otu tiles;
                # broadcast them across 64 partitions via K=1 matmuls
                pull(1)  # stream a projection run while the DVE chain runs
                rps = psW.tile([P, QT_TILE], F32, tag="w", name="rps")
                for h in range(2):
                    nc.tensor.matmul(
                        rps[h * HD : (h + 1) * HD, :],
                        onesf_sb[HD : HD + 1, :],
                        otu[h][HD : HD + 1, :],
                        start=True, stop=True,
                    )
                rsum_sb = sm.tile([HD, 2 * QT_TILE], F32, tag="rsum")
                nc.vector.tensor_copy(out=rsum_sb[:, 0:QT_TILE], in_=rps[0:HD, :])
                nc.vector.tensor_copy(out=rsum_sb[:, QT_TILE:], in_=rps[HD:CPC, :])
                r_sb = sm.tile([HD, 2 * QT_TILE], F32, tag="rsb")
                nc.vector.reciprocal_approx_fast(out=r_sb[:], in_=rsum_sb[:])
                on = otn_pool.tile([P, QT_TILE], CD, tag="otn")
                nc.vector.tensor_mul(
                    out=on[0:HD, :], in0=otu[0][0:HD, :], in1=r_sb[:, 0:QT_TILE]
                )
                nc.vector.tensor_mul(
                    out=on[HD:CPC, :], in0=otu[1][0:HD, :], in1=r_sb[:, QT_TILE:]
                )
                pull(1)
                # output projection: y[q0:q0+512, :] partial = on.T @ wo
                for j in range(QT_TILE // P):
                    ysb = yp.tile([P, D], F32, tag="y")
                    for e in range(D // QT_TILE):
                        yps = psOT.tile([P, QT_TILE], F32, tag="ot", name="yps")
                        nc.tensor.matmul(
                            yps[:],
                            on[:, j * P : (j + 1) * P],
                            wo_sb[:, e * QT_TILE : (e + 1) * QT_TILE],
                            start=True, stop=True,
                        )
                        nc.vector.tensor_copy(
                            out=ysb[:, e * QT_TILE : (e + 1) * QT_TILE], in_=yps[:]
                        )
                    nc.sync.dma_start(y[q0 + j * P : q0 + (j + 1) * P, :], ysb[:])

            blocks = [(b, qa) for b in range(B) for qa in range(NQA)]
            pending = []
            for bi, (b, qa) in enumerate(blocks):
                # all of batch b's projections must be emitted before its
                # attention reads them (deps are traced in emission order)
                deficit = UNITS_PER_CHUNK * (b + 1) - pulled[0]
                if deficit > 0:
                    pull(deficit)
                pts = {0: emit_st_exp(b, qa, 0), 1: emit_st_exp(b, qa, 1)}
                if len(pending) >= 2:
                    finalize(pending.pop(0))
                ot = [
                    psOT.tile([P, QT_TILE], F32, tag="ot", name=f"ot{h}")
                    for h in range(2)
                ]
                for kt in range(2, NKT, 2):
                    pts[kt] = emit_st_exp(b, qa, kt)
                    pts[kt + 1] = emit_st_exp(b, qa, kt + 1)
                    if kt % 6 == 2:
                        pull(1)
                    emit_av_group(
                        ot, b, (kt - 2, kt - 1),
                        {kt - 2: pts.pop(kt - 2), kt - 1: pts.pop(kt - 1)},
                    )
                emit_av_group(
                    ot, b, (NKT - 2, NKT - 1),
                    {NKT - 2: pts.pop(NKT - 2), NKT - 1: pts.pop(NKT - 1)},
                )
                # evacuate OT psum to SBUF immediately: frees the psum banks
                # and takes the normalization chain off the critical path
                otu = [
                    otu_pool.tile([HD + 1, QT_TILE], F32, tag="otu", name=f"otu{h}")
                    for h in range(2)
                ]
                for h in range(2):
                    nc.vector.tensor_copy(out=otu[h][:], in_=ot[h][0 : HD + 1, :])
                pending.append((b, qa, otu))
            for fin in pending:
                finalize(fin)

    nc.compile()
    _NC_CACHE["nc"] = nc
    return nc


def make_in_maps(inputs):
    x = np.asarray(inputs["x"], np.float32)
    Wq = np.asarray(inputs["Wq"], np.float32)
    Wk = np.asarray(inputs["Wk"], np.float32)
    Wv = np.asarray(inputs["Wv"], np.float32)
    Wo = np.asarray(inputs["Wo"], np.float32)
    bq = np.asarray(inputs["bq"], np.float32)
    bk = np.asarray(inputs["bk"], np.float32)
    bv = np.asarray(inputs["bv"], np.float32)

    xT = np.ascontiguousarray(x.reshape(BS, D).T).astype(CD_NP)
    in_maps = []
    for c in range(NCORES):
        sl = slice(c * CPC, (c + 1) * CPC)
        in_maps.append(
            {
                "xT": xT,
                "wq": np.ascontiguousarray(Wq[:, sl]).astype(CD_NP),
                "wk": np.ascontiguousarray(Wk[:, sl]).astype(CD_NP),
                "wv": np.ascontiguousarray(Wv[:, sl]).astype(CD_NP),
                "wo": np.ascontiguousarray(Wo[sl, :]).astype(CD_NP),
                "bq": np.ascontiguousarray(bq[sl].reshape(1, CPC)).astype(CD_NP),
                "bk": np.ascontiguousarray(bk[sl].reshape(1, CPC)).astype(CD_NP),
                "bv": np.ascontiguousarray(bv[sl].reshape(1, CPC)).astype(CD_NP),
            }
        )
    return in_maps


def kernel(**inputs):
    global LAST_RESULTS
    bo = np.asarray(inputs["bo"], np.float32)
    nc = build_nc()
    in_maps = make_in_maps(inputs)
    res = bass_utils.run_bass_kernel_spmd(nc, in_maps, core_ids=list(range(NCORES)))
    LAST_RESULTS = res
    acc = np.zeros((BS, D), np.float64)
    for r in res.results:
        acc += r["y"].astype(np.float64)
    out = (acc + bo.astype(np.float64)).astype(np.float32)
    return out.reshape(B, S, D)


# revision 22
# speedup vs baseline: 1.9558x; 1.0163x over previous
"""Multi-head attention (B=4, S=2048, D=1024, H=16, Hd=64) on 8 TRN2 NeuronCores.

Sharding: tensor-parallel over heads — 2 heads per core (128 channels).
Each core computes its heads' Q/K/V projections, attention, and the partial
output projection (its 128 rows of Wo); the host sums the 8 partials + bo.

Device-side structure (per core):
  - x is pre-transposed on host to xT [D, B*S]; streamed in 4 big chunks.
  - Q, K produced transposed: QT/KT [128ch, B*S], heads stacked on
    partitions (head0 rows 0:64, head1 rows 64:128). The two heads' K=64
    score matmuls are emitted adjacently at disjoint row groups, so they
    run concurrently in the 128x128 PE array.
  - V is computed transposed (VT, N=512 matmuls) then PE-transposed into
    natural [seq, ch] layout with a ones-column per head; the attention
    output matmul OT[65, q] = V_aug.T @ P carries the softmax denominator
    in row 64 for free.
  - Both heads' score tiles share one 2-bank PSUM tile, so exp() runs as
    a single 1024-wide ACT op (half the ACT instruction count).
  - Attention is software-pipelined: AV matmuls lag the score matmuls by
    2 k-steps, and the normalization + output projection of block i is
    emitted inside block i+1's first score matmuls, so the PE never
    stalls on the ACT/DVE chains.
  - No max-subtraction in softmax: scores ~ N(0,1) by construction
    (|score| < ~7), exp() is safe in fp32.
"""
import sys

sys.path.insert(0, "/opt/trn_rl_repo")

import numpy as np
import ml_dtypes

import concourse.bass as bass
import concourse.mybir as mybir
import concourse.tile as tile
from concourse import bacc, bass_utils
from concourse.masks import make_identity

B, S, D = 4, 2048, 1024
BS = B * S            # 8192 rows
NCORES = 8
CPC = 128             # channels per core (2 heads x 64)
HD = 64               # head dim
P = 128
QT_TILE = 512         # q-tile width
NQT = BS // QT_TILE   # 16
NKT = S // P          # 16 k-tiles per batch
NQA = S // QT_TILE    # 4 q-tiles per batch

F32 = mybir.dt.float32
CD = mybir.dt.bfloat16          # compute dtype on device
CD_NP = ml_dtypes.bfloat16

LAST_RESULTS = None
_NC_CACHE = {}


def build_nc():
    if "nc" in _NC_CACHE:
        return _NC_CACHE["nc"]
    nc = bacc.Bacc(trn_type="TRN2", num_devices=NCORES)

    xT = nc.dram_tensor("xT", [D, BS], CD, kind="ExternalInput").ap()
    wq = nc.dram_tensor("wq", [D, CPC], CD, kind="ExternalInput").ap()
    wk = nc.dram_tensor("wk", [D, CPC], CD, kind="ExternalInput").ap()
    wv = nc.dram_tensor("wv", [D, CPC], CD, kind="ExternalInput").ap()
    wo = nc.dram_tensor("wo", [CPC, D], CD, kind="ExternalInput").ap()
    bq = nc.dram_tensor("bq", [CPC, 1], F32, kind="ExternalInput").ap()
    bk = nc.dram_tensor("bk", [CPC, 1], F32, kind="ExternalInput").ap()
    bv = nc.dram_tensor("bv", [CPC, 1], F32, kind="ExternalInput").ap()
    y = nc.dram_tensor("y", [BS, D], F32, kind="ExternalOutput").ap()

    KCH = D // P  # 8 contraction chunks for the projections
    scale = float(1.0 / np.sqrt(np.float32(HD)))

    with tile.TileContext(nc) as tc:
        with (
            tc.tile_pool(name="pers", bufs=1) as pers,
            tc.tile_pool(name="xin", bufs=2) as xin,
            tc.tile_pool(name="vtp", bufs=2) as vtp,
            tc.tile_pool(name="pt", bufs=4) as pt,
            tc.tile_pool(name="otn", bufs=2) as otn_pool,
            tc.tile_pool(name="yp", bufs=3) as yp,
            tc.tile_pool(name="sm", bufs=4) as sm,
            tc.tile_pool(name="otu", bufs=4) as otu_pool,
            tc.tile_pool(name="psW", bufs=2, space="PSUM") as psW,
            tc.tile_pool(name="psOT", bufs=2, space="PSUM") as psOT,
            tc.tile_pool(name="ps2", bufs=2, space="PSUM") as ps2,
        ):
            # ---- persistent tensors ----
            qt_sb = pers.tile([P, BS], CD, tag="QT")
            kt_sb = pers.tile([P, BS], CD, tag="KT")
            v_sb = pers.tile([P, BS // P, 2 * HD + 2], CD, tag="V")
            wq_sb = pers.tile([P, KCH, CPC], CD, tag="wq")
            wk_sb = pers.tile([P, KCH, CPC], CD, tag="wk")
            wv_sb = pers.tile([P, KCH, CPC], CD, tag="wv")
            wo_sb = pers.tile([P, D], CD, tag="wo")
            bq_sb = pers.tile([CPC, 1], F32, tag="bq")
            bk_sb = pers.tile([CPC, 1], F32, tag="bk")
            bv_sb = pers.tile([CPC, 1], F32, tag="bv")
            ones_sb = pers.tile([1, QT_TILE], CD, tag="ones")
            onesf_sb = pers.tile([P, HD], F32, tag="onesf")
            ident_sb = pers.tile([P, P], CD, tag="ident")

            nc.sync.dma_start(wq_sb[:], wq.rearrange("(o p) c -> p o c", p=P))
            nc.sync.dma_start(wk_sb[:], wk.rearrange("(o p) c -> p o c", p=P))
            nc.sync.dma_start(wv_sb[:], wv.rearrange("(o p) c -> p o c", p=P))
            nc.sync.dma_start(wo_sb[:], wo[:, :])
            nc.sync.dma_start(bq_sb[:], bq[:, :])
            nc.sync.dma_start(bk_sb[:], bk[:, :])
            nc.sync.dma_start(bv_sb[:], bv[:, :])
            nc.vector.memset(ones_sb[:], 1.0)
            nc.vector.memset(onesf_sb[:], 1.0)
            make_identity(nc, ident_sb[:])

            # ---- phase 1: projections, as a lazily-driven generator ----
            # Units are pulled from inside the attention loop so projection
            # matmuls (pure PE) fill the PE idle left by ACT-paced attention.
            XQ = BS // 4  # 2048 rows per x chunk

            def proj_gen():
                for xq in range(4):
                    xt = xin.tile([P, KCH, XQ], CD, tag="xt")
                    nc.sync.dma_start(
                        xt[:],
                        xT[:, xq * XQ : (xq + 1) * XQ].rearrange(
                            "(o p) q -> p o q", p=P
                        ),
                    )
                    yield
                    for lq in range(XQ // QT_TILE):
                        q0 = xq * XQ + lq * QT_TILE
                        l0 = lq * QT_TILE
                        for w_sb, b_sb, dst in (
                            (wq_sb, bq_sb, qt_sb),
                            (wk_sb, bk_sb, kt_sb),
                            (wv_sb, bv_sb, None),
                        ):
                            # one unit = one full 9-matmul accumulation run:
                            # consecutive same-bank matmuls stream at N/2.4;
                            # splitting a run across units breaks that.
                            pj = psW.tile([P, QT_TILE], F32, tag="w", name="pj")
                            for o in range(KCH):
                                nc.tensor.matmul(
                                    pj[:], w_sb[:, o, :], xt[:, o, l0 : l0 + QT_TILE],
                                    start=(o == 0), stop=(o == KCH - 1),
                                )
                            if dst is not None:
                                nc.vector.tensor_scalar_add(
                                    dst[:, q0 : q0 + QT_TILE], pj[:], b_sb[:, 0:1]
                                )
                                yield
                            else:
                                # V: VT chunk -> PE-transpose into natural
                                # layout with per-head ones-columns.
                                vt_sb = vtp.tile([P, QT_TILE], CD, tag="vt")
                                nc.vector.tensor_scalar_add(
                                    vt_sb[:], pj[:], b_sb[:, 0:1]
                                )
                                yield
                                for rt in range(QT_TILE // P):
                                    tp = psW.tile([P, P], CD, tag="w", name="tp")
                                    nc.tensor.transpose(
                                        tp[:], vt_sb[:, rt * P : (rt + 1) * P],
                                        ident_sb[:],
                                    )
                                    grt = q0 // P + rt
                                    nc.vector.tensor_copy(
                                        out=v_sb[:, grt, 0:HD], in_=tp[:, 0:HD]
                                    )
                                    nc.vector.tensor_copy(
                                        out=v_sb[:, grt, HD + 1 : 2 * HD + 1],
                                        in_=tp[:, HD:CPC],
                                    )
                                    nc.vector.memset(v_sb[:, grt, HD : HD + 1], 1.0)
                                    nc.vector.memset(
                                        v_sb[:, grt, 2 * HD + 1 : 2 * HD + 2], 1.0
                                    )
                                yield

            gen = proj_gen()
            pulled = [0]

            def pull(n):
                for _ in range(n):
                    if next(gen, "done") == "done":
                        break
                    pulled[0] += 1

            UNITS_PER_CHUNK = 1 + 4 * 4
            pull(UNITS_PER_CHUNK)  # batch 0's projections up front

            # ---- phase 2+3: attention + output projection ----
            def emit_st_exp(b, qa, kt):
                q0 = b * S + qa * QT_TILE
                k0 = b * S + kt * P
                stp = ps2.tile([P, 2 * QT_TILE], F32, tag="stp", name="stp")
                for h in range(2):
                    hp = h * HD
                    nc.tensor.matmul(
                        stp[:, h * QT_TILE : (h + 1) * QT_TILE],
                        kt_sb[hp : hp + HD, k0 : k0 + P],
                        qt_sb[hp : hp + HD, q0 : q0 + QT_TILE],
                        start=True, stop=True,
                    )
                p_t = pt.tile([P, 2 * QT_TILE], CD, tag="p", name="p")
                nc.scalar.activation(
                    p_t[:], stp[:], mybir.ActivationFunctionType.Exp, scale=scale
                )
                return p_t

            def emit_av_group(ot, b, kts, ptd):
                # per head, run all kts back-to-back into the same OT bank
                # (same-bank accumulation streams on the PE)
                for h in range(2):
                    vcol = h * (HD + 1)
                    for kt in kts:
                        nc.tensor.matmul(
                            ot[h][0 : HD + 1, :],
                            v_sb[:, b * NKT + kt, vcol : vcol + HD + 1],
                            ptd[kt][:, h * QT_TILE : (h + 1) * QT_TILE],
                            start=(kt == 0), stop=(kt == NKT - 1),
                        )

            def finalize(fin):
                b, qa, otu = fin
                q0 = b * S + qa * QT_TILE
                # denominators live in row 64 of the evacuated otu tiles;
                # broadcast them across 64 partitions via K=1 matmuls
                pull(1)  # stream a projection run while the DVE chain runs
                rps = psW.tile([P, QT_TILE], F32, tag="w", name="rps")
                for h in range(2):
                    nc.tensor.matmul(
                        rps[h * HD : (h + 1) * HD, :],
                        onesf_sb[HD : HD + 1, :],
                        otu[h][HD : HD + 1, :],
                        start=True, stop=True,
                    )
                rsum_sb = sm.tile([HD, 2 * QT_TILE], F32, tag="rsum")
                nc.vector.tensor_copy(out=rsum_sb[:, 0:QT_TILE], in_=rps[0:HD, :])
                nc.vector.tensor_copy(out=rsum_sb[:, QT_TILE:], in_=rps[HD:CPC, :])
                r_sb = sm.tile([HD, 2 * QT_TILE], F32, tag="rsb")
                nc.vector.reciprocal_approx_fast(out=r_sb[:], in_=rsum_sb[:])
                on = otn_pool.tile([P, QT_TILE], CD, tag="otn")
                nc.vector.tensor_mul(
                    out=on[0:HD, :], in0=otu[0][0:HD, :], in1=r_sb[:, 0:QT_TILE]
                )
                nc.vector.tensor_mul(
                    out=on[HD:CPC, :], in0=otu[1][0:HD, :], in1=r_sb[:, QT_TILE:]
                )
                pull(1)
                # output projection: y[q0:q0+512, :] partial = on.T @ wo
                for j in range(QT_TILE // P):
                    ysb = yp.tile([P, D], F32, tag="y")
                    for e in range(D // QT_TILE):
                        yps = psOT.tile([P, QT_TILE], F32, tag="ot", name="yps")
                        nc.tensor.matmul(
                            yps[:],
                            on[:, j * P : (j + 1) * P],
                            wo_sb[:, e * QT_TILE : (e + 1) * QT_TILE],
                            start=True, stop=True,
                        )
                        nc.vector.tensor_copy(
                            out=ysb[:, e * QT_TILE : (e + 1) * QT_TILE], in_=yps[:]
                        )
                    nc.sync.dma_start(y[q0 + j * P : q0 + (j + 1) * P, :], ysb[:])

            blocks = [(b, qa) for b in range(B) for qa in range(NQA)]
            pending = []
            for bi, (b, qa) in enumerate(blocks):
                # all of batch b's projections must be emitted before its
                # attention reads them (deps are traced in emission order)
                deficit = UNITS_PER_CHUNK * (b + 1) - pulled[0]
                if deficit > 0:
                    pull(deficit)
                pts = {0: emit_st_exp(b, qa, 0), 1: emit_st_exp(b, qa, 1)}
                if len(pending) >= 2:
                    finalize(pending.pop(0))
                ot = [
                    psOT.tile([P, QT_TILE], F32, tag="ot", name=f"ot{h}")
                    for h in range(2)
                ]
                for kt in range(2, NKT, 2):
                    pts[kt] = emit_st_exp(b, qa, kt)
                    pts[kt + 1] = emit_st_exp(b, qa, kt + 1)
                    if kt % 4 == 2:
                        pull(1)
                    emit_av_group(
                        ot, b, (kt - 2, kt - 1),
                        {kt - 2: pts.pop(kt - 2), kt - 1: pts.pop(kt - 1)},
                    )
                emit_av_group(
                    ot, b, (NKT - 2, NKT - 1),
                    {NKT - 2: pts.pop(NKT - 2), NKT - 1: pts.pop(NKT - 1)},
                )
                # evacuate OT psum to SBUF immediately: frees the psum banks
                # and takes the normalization chain off the critical path
                otu = [
                    otu_pool.tile([HD + 1, QT_TILE], F32, tag="otu", name=f"otu{h}")
                    for h in range(2)
                ]
                for h in range(2):
                    nc.vector.tensor_copy(out=otu[h][:], in_=ot[h][0 : HD + 1, :])
                pending.append((b, qa, otu))
            for fin in pending:
                finalize(fin)

    nc.compile()
    _NC_CACHE["nc"] = nc
    return nc


def make_in_maps(inputs):
    x = np.asarray(inputs["x"], np.float32)
    Wq = np.asarray(inputs["Wq"], np.float32)
    Wk = np.asarray(inputs["Wk"], np.float32)
    Wv = np.asarray(inputs["Wv"], np.float32)
    Wo = np.asarray(inputs["Wo"], np.float32)
    bq = np.asarray(inputs["bq"], np.float32)
    bk = np.asarray(inputs["bk"], np.float32)
    bv = np.asarray(inputs["bv"], np.float32)

    xT = np.ascontiguousarray(x.reshape(BS, D).T).astype(CD_NP)
    in_maps = []
    for c in range(NCORES):
        sl = slice(c * CPC, (c + 1) * CPC)
        in_maps.append(
            {
                "xT": xT,
                "wq": np.ascontiguousarray(Wq[:, sl]).astype(CD_NP),
                "wk": np.ascontiguousarray(Wk[:, sl]).astype(CD_NP),
                "wv": np.ascontiguousarray(Wv[:, sl]).astype(CD_NP),
                "wo": np.ascontiguousarray(Wo[sl, :]).astype(CD_NP),
                "bq": np.ascontiguousarray(bq[sl].reshape(CPC, 1)),
                "bk": np.ascontiguousarray(bk[sl].reshape(CPC, 1)),
                "bv": np.ascontiguousarray(bv[sl].reshape(CPC, 1)),
            }
        )
    return in_maps


def kernel(**inputs):
    global LAST_RESULTS
    bo = np.asarray(inputs["bo"], np.float32)
    nc = build_nc()
    in_maps = make_in_maps(inputs)
    res = bass_utils.run_bass_kernel_spmd(nc, in_maps, core_ids=list(range(NCORES)))
    LAST_RESULTS = res
    acc = np.zeros((BS, D), np.float64)
    for r in res.results:
        acc += r["y"].astype(np.float64)
    out = (acc + bo.astype(np.float64)).astype(np.float32)
    return out.reshape(B, S, D)
